# revision 1
# baseline (speedup 1.0000x reference)
"""CrossModalMoELayer Trainium2 Bass kernel.

Sharding: data-parallel over batch B=8 across the 8 NeuronCores (one batch
element per core). Each core runs the full layer for its batch element:
self-attention, cross-attention, gating, and the dense 8-expert MoE
(weights replicated, streamed from HBM).

Layouts on device:
  feature-major ("fm"): [feat_part=128, feat_chunk, tokens]  - activations
  All matmuls run as float32r (full-rate fp32 PE mode, fp32 PSUM accumulate).

kernel(**inputs) takes the FULL unsharded inputs (numpy, keyed as in
setup_inputs()) and returns the full (query_tokens, image_tokens) tuple.
"""

import numpy as np

import concourse.bass as bass
import concourse.tile as tile
from concourse import bacc, mybir
from concourse.bass_utils import run_bass_kernel_spmd
from concourse.masks import make_identity

B, T, H, NH, HD, F, E = 8, 256, 1024, 16, 64, 4096, 8
IC = H // 128          # 8 feature chunks of the model dim
FT = F // 128          # 32 feature chunks of the FFN dim
T2 = 2 * T             # query tokens + image tokens concatenated
EPS = 1e-5

F32 = mybir.dt.float32
F32R = mybir.dt.float32r
AX = mybir.AxisListType
ALU = mybir.AluOpType
AF = mybir.ActivationFunctionType


def _r(ap):
    return ap.bitcast(F32R)


# ----------------------------------------------------------------------------
# program builder
# ----------------------------------------------------------------------------

def _build_program():
    nc = bacc.Bacc(
        "TRN2",
        target_bir_lowering=False,
        debug=False,
        enable_asserts=False,
        num_devices=8,
    )

    dt = {}

    def din(name, shape, d=F32):
        dt[name] = nc.dram_tensor(name, list(shape), d, kind="ExternalInput").ap()
        return dt[name]

    def dout(name, shape):
        dt[name] = nc.dram_tensor(name, list(shape), F32, kind="ExternalOutput").ap()
        return dt[name]

    # activations (per core)
    din("xq", (128, IC, T), F32R)
    din("xi", (128, IC, T), F32R)
    din("xt", (128, IC, T), F32R)
    # attention weights: [proj, ot, i(128), ic, o(128)]
    din("w_sa", (3, 8, 128, IC, 128), F32R)
    din("b_sa", (128, 3, IC))
    din("w_sao", (8, 128, IC, 128), F32R)
    din("b_sao", (128, IC))
    din("w_ca", (3, 8, 128, IC, 128), F32R)
    din("b_ca", (128, 3, IC))
    din("w_cao", (8, 128, IC, 128), F32R)
    din("b_cao", (128, IC))
    # gates
    din("w_ig1", (128, IC, E), F32R)
    din("w_ig2", (128, IC, E), F32R)
    din("b_ig", (1, E))
    din("w_tg1", (128, IC, E), F32R)
    din("w_tg2", (128, IC, E), F32R)
    din("b_tg", (1, E))
    # layernorms [128, IC]
    for n in ("g_lnq", "b_lnq", "g_lnc", "b_lnc", "g_lnf", "b_lnf"):
        din(n, (128, IC))
    # experts
    din("w1", (E, FT, 128, IC, 128), F32R)     # [e, ft, i, ic, f]
    din("b1", (128, E, FT))
    din("w2", (E, 2, FT, 128, 512), F32R)      # [e, og, ft, f, o]
    din("b2", (128, E, IC))
    # outputs [p(=t within tile), tt, o]
    dout("oq", (128, 2, H))
    dout("oi", (128, 2, H))

    with tile.TileContext(nc) as tc:
        _trace_kernel(nc, tc, dt)

    nc.compile()
    return nc


def _trace_kernel(nc, tc, dt):
    persist = tc.alloc_tile_pool(name="persist", bufs=1)

    # ---- constants + small params --------------------------------------
    ident = persist.tile([128, 128], F32, tag="ident")
    make_identity(nc, ident)
    ones_f = persist.tile([128, 1], F32, tag="ones_f")
    nc.vector.memset(ones_f, 1.0)
    ones = persist.tile([128, 1], F32R, tag="ones")
    nc.vector.tensor_copy(ones, ones_f)
    identr = persist.tile([128, 128], F32R, tag="identr")
    nc.vector.tensor_copy(identr, ident)
    eps_t = persist.tile([1, 1], F32, tag="eps")
    nc.vector.memset(eps_t, EPS)

    def load(name, shape, d=F32, pool=persist):
        t = pool.tile(list(shape), d, tag=f"ld_{name}")
        nc.sync.dma_start(out=t, in_=dt[name])
        return t

    xi0 = load("xi", (128, IC, T), F32R)
    xt0 = load("xt", (128, IC, T), F32R)
    b_sa = load("b_sa", (128, 3, IC))
    b_sao = load("b_sao", (128, IC))
    b_ca = load("b_ca", (128, 3, IC))
    b_cao = load("b_cao", (128, IC))
    w_ig1 = load("w_ig1", (128, IC, E), F32R)
    w_ig2 = load("w_ig2", (128, IC, E), F32R)
    b_ig = load("b_ig", (1, E))
    w_tg1 = load("w_tg1", (128, IC, E), F32R)
    w_tg2 = load("w_tg2", (128, IC, E), F32R)
    b_tg = load("b_tg", (1, E))
    lnp = {n: load(n, (128, IC)) for n in
           ("g_lnq", "b_lnq", "g_lnc", "b_lnc", "g_lnf", "b_lnf")}
    b1f = load("b1", (128, E, FT))
    b2f = load("b2", (128, E, IC))

    # persistent activations
    q2 = persist.tile([128, IC, T], F32R, tag="q2")          # query after CA
    x_moe = persist.tile([128, IC, T2], F32R, tag="x_moe")   # [lnf(q2) ; xi0]
    probs_bc = persist.tile([128, E, T2], F32, tag="probs") # router probs bcast
    acc = persist.tile([128, IC, T2], F32, tag="acc")       # MoE accumulator

    dram = tc.alloc_tile_pool(name="dram", bufs=1, space="DRAM")
    scr_probs = dram.tile([2, E, T], F32, tag="scr_probs")

    # ====================================================================
    # phase 1: attention + gating + lnf (own pools, released before MoE)
    # ====================================================================
    aps_mm = tc.alloc_tile_pool(name="aps_mm", bufs=3, space="PSUM")
    aps_tr = tc.alloc_tile_pool(name="aps_tr", bufs=2, space="PSUM")
    aps_pv = tc.alloc_tile_pool(name="aps_pv", bufs=2, space="PSUM")
    aps_sm = tc.alloc_tile_pool(name="aps_sm", bufs=1, space="PSUM")
    awork = tc.alloc_tile_pool(name="awork", bufs=2)
    aw1 = tc.alloc_tile_pool(name="aw1", bufs=1)
    wpool = tc.alloc_tile_pool(name="wpool", bufs=4)

    def ln_fm(dst, src, g, b):
        """dst[:, ic, :] = LN over features of src (fm layout [128, IC, T])."""
        ntok = src.shape[2]
        sum_ps = aps_sm.tile([1, ntok], F32, tag="sm")
        for ic in range(IC):
            nc.tensor.matmul(sum_ps, ones, src[:, ic, :],
                             start=(ic == 0), stop=(ic == IC - 1))
        mean = awork.tile([1, ntok], F32, tag="ln_mean")
        nc.scalar.mul(mean, sum_ps, 1.0 / H)
        sumsq_ps = aps_sm.tile([1, ntok], F32, tag="sm")
        for ic in range(IC):
            xsq = awork.tile([128, ntok], F32R, tag="ln_xsq")
            nc.scalar.activation(xsq, src[:, ic, :], AF.Square)
            nc.tensor.matmul(sumsq_ps, ones, xsq,
                             start=(ic == 0), stop=(ic == IC - 1))
        msq = awork.tile([1, ntok], F32, tag="ln_msq")
        nc.vector.tensor_mul(msq, mean, mean)
        var = awork.tile([1, ntok], F32, tag="ln_var")
        nc.vector.scalar_tensor_tensor(var, in0=sumsq_ps, scalar=1.0 / H,
                                       in1=msq, op0=ALU.mult, op1=ALU.subtract)
        std = awork.tile([1, ntok], F32, tag="ln_std")
        nc.scalar.activation(std, var, AF.Sqrt, bias=eps_t)
        rstd = awork.tile([1, ntok], F32, tag="ln_rstd")
        nc.vector.reciprocal(rstd, std)
        negc = awork.tile([1, ntok], F32, tag="ln_negc")
        nc.vector.scalar_tensor_tensor(negc, in0=mean, scalar=-1.0,
                                       in1=rstd, op0=ALU.mult, op1=ALU.mult)
        a_bc = awork.tile([128, ntok], F32, tag="ln_abc")
        nc.gpsimd.partition_broadcast(a_bc, rstd)
        c_bc = awork.tile([128, ntok], F32, tag="ln_cbc")
        nc.gpsimd.partition_broadcast(c_bc, negc)
        for ic in range(IC):
            nc.vector.tensor_mul(dst[:, ic, :], src[:, ic, :], a_bc)
            nc.vector.tensor_add(dst[:, ic, :], dst[:, ic, :], c_bc)
            nc.vector.tensor_scalar(out=dst[:, ic, :], in0=dst[:, ic, :],
                                    scalar1=g[:, ic:ic + 1], scalar2=b[:, ic:ic + 1],
                                    op0=ALU.mult, op1=ALU.add)

    def proj_fm(dst, src, w_dram_ot, bias, bias_col):
        """dst[:, ot, :] = W @ src + b  (fm in, fm out).

        w_dram_ot(ot) -> DRAM AP [128, IC, 128]; bias[:, bias_col(ot)] is the
        per-partition bias column [128, 1].
        """
        ntok = src.shape[2]
        for ot in range(IC):
            wt = wpool.tile([128, IC, 128], F32R, tag="wsl")
            nc.sync.dma_start(out=wt, in_=w_dram_ot(ot))
            ps = aps_mm.tile([128, ntok], F32, tag="mm")
            for ic in range(IC):
                nc.tensor.matmul(ps, wt[:, ic, :], src[:, ic, :],
                                 start=(ic == 0), stop=(ic == IC - 1))
            nc.scalar.add(dst[:, ot, :], ps, bias[:, bias_col(ot)])

    def attention(new_resid, old_resid, qsrc, kvsrc, w_in, b_in, w_out, b_out):
        """new_resid = old_resid + out_proj(MHA(q=qsrc, kv=kvsrc)); all fm."""
        qf = aw1.tile([128, IC, T], F32R, tag="qf")
        kf = aw1.tile([128, IC, T], F32R, tag="kf")
        vf = aw1.tile([128, IC, T], F32R, tag="vf")
        proj_fm(qf, qsrc, lambda ot: w_in[0, ot], b_in, lambda ot: slice(0 * IC + ot, 0 * IC + ot + 1))
        proj_fm(kf, kvsrc, lambda ot: w_in[1, ot], b_in, lambda ot: slice(1 * IC + ot, 1 * IC + ot + 1))
        proj_fm(vf, kvsrc, lambda ot: w_in[2, ot], b_in, lambda ot: slice(2 * IC + ot, 2 * IC + ot + 1))
        # attention output, token-major: ao_tm[t(128), qt, h*64+d]
        ao_tm = aw1.tile([128, 2, H], F32R, tag="ao_tm")
        for pair in range(NH // 2):
            per_head = []
            for h in (2 * pair, 2 * pair + 1):
                base = (h % 2) * HD
                c = h // 2
                qh = qf[base:base + HD, c, :]
                kh = kf[base:base + HD, c, :]
                vh = vf[base:base + HD, c, :]
                idn = identr[base:base + HD, base:base + HD]
                # vh^T : [T, HD] in two 128-token tiles
                vht = awork.tile([128, 2, HD], F32R, tag="vht",
                                 name=f"vht_{h}")
                for kt in range(2):
                    tp = aps_tr.tile([128, HD], F32R, tag="tr")
                    nc.tensor.transpose(tp, vh[:, kt * 128:(kt + 1) * 128], idn)
                    nc.vector.tensor_copy(vht[:, kt, :], tp)
                attn_t = awork.tile([128, 2, T], F32R, tag="attnT",
                                    name=f"attnT_{h}")
                for qt in range(2):
                    sc = aps_mm.tile([128, T], F32, tag="mm")
                    nc.tensor.matmul(sc, qh[:, qt * 128:(qt + 1) * 128], kh,
                                     start=True, stop=True)
                    nmax = awork.tile([128, 1], F32, tag="nmax")
                    nc.vector.reduce_max(nmax, sc, axis=AX.X, negate=True)
                    nmax2 = awork.tile([128, 1], F32, tag="nmax2")
                    nc.scalar.mul(nmax2, nmax, 0.125)
                    asb = awork.tile([128, T], F32, tag="asb")
                    ssum = awork.tile([128, 1], F32, tag="ssum")
                    nc.scalar.activation(asb, sc, AF.Exp, bias=nmax2, scale=0.125,
                                         accum_out=ssum)
                    rsum = awork.tile([128, 1], F32, tag="rsum")
                    nc.vector.reciprocal(rsum, ssum)
                    asb_r = awork.tile([128, T], F32R, tag="asb_r")
                    nc.vector.tensor_scalar_mul(asb_r, asb, rsum)
                    for kt in range(2):
                        tp2 = aps_tr.tile([128, 128], F32R, tag="tr")
                        nc.tensor.transpose(tp2, asb_r[:, kt * 128:(kt + 1) * 128],
                                            identr)
                        nc.vector.tensor_copy(
                            attn_t[:, kt, qt * 128:(qt + 1) * 128], tp2)
                per_head.append((vht, attn_t))
            # PV for the pair, token-major: out[q, d] per qt into one psum tile
            for qt in range(2):
                pvp = aps_pv.tile([128, 2 * HD], F32, tag="pv")
                for j, (vht, attn_t) in enumerate(per_head):
                    for kt in range(2):
                        nc.tensor.matmul(pvp[:, j * HD:(j + 1) * HD],
                                         attn_t[:, kt, qt * 128:(qt + 1) * 128],
                                         vht[:, kt, :],
                                         start=(kt == 0), stop=(kt == 1))
                nc.scalar.copy(ao_tm[:, qt, pair * 2 * HD:(pair + 1) * 2 * HD], pvp)
        # transpose ao back to feature-major for the output projection
        ao = aw1.tile([128, IC, T], F32R, tag="ao")
        for oc in range(IC):
            for qt in range(2):
                tpo = aps_tr.tile([128, 128], F32R, tag="tr")
                nc.tensor.transpose(tpo, ao_tm[:, qt, oc * 128:(oc + 1) * 128],
                                    identr)
                nc.vector.tensor_copy(ao[:, oc, qt * 128:(qt + 1) * 128], tpo)
        # out-proj + bias + residual
        for ot in range(IC):
            wt = wpool.tile([128, IC, 128], F32R, tag="wsl")
            nc.sync.dma_start(out=wt, in_=w_out[ot])
            ps = aps_mm.tile([128, T], F32, tag="mm")
            for ic in range(IC):
                nc.tensor.matmul(ps, wt[:, ic, :], ao[:, ic, :],
                                 start=(ic == 0), stop=(ic == IC - 1))
            nc.vector.scalar_tensor_tensor(new_resid[:, ot, :], in0=ps,
                                           scalar=b_out[:, ot:ot + 1],
                                           in1=old_resid[:, ot, :],
                                           op0=ALU.add, op1=ALU.add)

    def gate(s, tokens_fm, w1sb, w2sb, bsb, ctx):
        """probs_bc[:, :, s*T:(s+1)*T] = softmax_E(tokens.W1 + ctx.W2 + b)."""
        ct_ps = aps_sm.tile([1, E], F32, tag="sm")
        for ic in range(IC):
            nc.tensor.matmul(ct_ps, ctx[:, ic, :], w2sb[:, ic, :],
                             start=(ic == 0), stop=(ic == IC - 1))
        crow = awork.tile([1, E], F32, tag="crow")
        nc.vector.tensor_add(crow, ct_ps, bsb)
        crow_bc = awork.tile([128, E], F32, tag="crow_bc")
        nc.gpsimd.partition_broadcast(crow_bc, crow)
        ptm = awork.tile([128, 2, E], F32, tag="ptm")
        for tt in range(2):
            lg_ps = aps_tr.tile([128, E], F32, tag="tr")
            for ic in range(IC):
                nc.tensor.matmul(lg_ps, tokens_fm[:, ic, tt * 128:(tt + 1) * 128],
                                 w1sb[:, ic, :],
                                 start=(ic == 0), stop=(ic == IC - 1))
            lg = awork.tile([128, E], F32, tag="lg")
            nc.vector.tensor_add(lg, lg_ps, crow_bc)
            nm = awork.tile([128, 1], F32, tag="gnm")
            nc.vector.reduce_max(nm, lg, axis=AX.X, negate=True)
            gs = awork.tile([128, 1], F32, tag="gs")
            nc.scalar.activation(ptm[:, tt, :], lg, AF.Exp, bias=nm, accum_out=gs)
            gr = awork.tile([128, 1], F32, tag="gr")
            nc.vector.reciprocal(gr, gs)
            nc.vector.tensor_scalar_mul(ptm[:, tt, :], ptm[:, tt, :], gr)
        pfm = awork.tile([E, 2, 128], F32, tag="pfm")
        for tt in range(2):
            tp = aps_tr.tile([E, 128], F32, tag="tr")
            nc.tensor.transpose(tp, ptm[:, tt, :], ident)
            nc.vector.tensor_copy(pfm[:, tt, :], tp)
        nc.sync.dma_start(out=scr_probs[s], in_=pfm)
        nc.sync.dma_start(out=probs_bc[:, :, s * T:(s + 1) * T],
                          in_=scr_probs[s].partition_broadcast(128))

    # ---- phase-1 body ---------------------------------------------------
    xq0 = aw1.tile([128, IC, T], F32R, tag="xq0")
    nc.sync.dma_start(out=xq0, in_=dt["xq"])

    qn = aw1.tile([128, IC, T], F32R, tag="qn")
    ln_fm(qn, xq0, lnp["g_lnq"], lnp["b_lnq"])
    q1 = aw1.tile([128, IC, T], F32R, tag="q1")
    attention(q1, xq0, qn, qn, dt["w_sa"], b_sa.rearrange("p a b -> p (a b)"),
              dt["w_sao"], b_sao)

    qn2 = aw1.tile([128, IC, T], F32R, tag="qn2")
    ln_fm(qn2, q1, lnp["g_lnc"], lnp["b_lnc"])
    attention(q2, q1, qn2, xi0, dt["w_ca"], b_ca.rearrange("p a b -> p (a b)"),
              dt["w_cao"], b_cao)

    # contexts: mean over tokens
    ictx = awork.tile([128, IC, 1], F32R, tag="ictx")
    tctx = awork.tile([128, IC, 1], F32R, tag="tctx")
    with nc.allow_low_precision(reason="f32r shares f32 bits; DVE sum is fp32"):
        for ic in range(IC):
            nc.vector.reduce_sum(ictx[:, ic, :], xi0[:, ic, :], axis=AX.X)
            nc.vector.reduce_sum(tctx[:, ic, :], xt0[:, ic, :], axis=AX.X)
    nc.scalar.mul(ictx.rearrange("p a b -> p (a b)"),
                  ictx.rearrange("p a b -> p (a b)"), 1.0 / T)
    nc.scalar.mul(tctx.rearrange("p a b -> p (a b)"),
                  tctx.rearrange("p a b -> p (a b)"), 1.0 / T)

    # routers: query stream uses txt gate on q2; image stream uses img gate
    gate(0, q2, w_tg1, w_tg2, b_tg, ictx)
    gate(1, xi0, w_ig1, w_ig2, b_ig, tctx)

    # moe input: [ lnf(q2) ; xi0 ]
    ln_fm(x_moe[:, :, 0:T], q2, lnp["g_lnf"], lnp["b_lnf"])
    nc.vector.tensor_copy(x_moe[:, :, T:T2], xi0)

    # moe accumulator initialised with the residuals
    nc.vector.tensor_copy(acc[:, :, 0:T], q2)
    nc.vector.tensor_copy(acc[:, :, T:T2], xi0)

    for p in (wpool, aw1, awork, aps_sm, aps_pv, aps_tr, aps_mm):
        p.release()

    # ====================================================================
    # phase 2: dense MoE over both streams (512 tokens)
    # ====================================================================
    mps_h = tc.alloc_tile_pool(name="mps_h", bufs=2, space="PSUM")
    mps_o = tc.alloc_tile_pool(name="mps_o", bufs=4, space="PSUM")
    hpool = tc.alloc_tile_pool(name="hpool", bufs=FT)
    mw1 = tc.alloc_tile_pool(name="mw1", bufs=4)
    mw2 = tc.alloc_tile_pool(name="mw2", bufs=6)
    mwork = tc.alloc_tile_pool(name="mwork", bufs=2)

    for e in range(E):
        h_tiles = []
        for ft in range(FT):
            w1t = mw1.tile([128, IC, 128], F32R, tag="w1sl")
            nc.sync.dma_start(out=w1t, in_=dt["w1"][e, ft])
            hps = mps_h.tile([128, T2], F32, tag="h")
            for ic in range(IC):
                nc.tensor.matmul(hps, w1t[:, ic, :], x_moe[:, ic, :],
                                 start=(ic == 0), stop=(ic == IC - 1))
            hsb = hpool.tile([128, T2], F32R, tag="h_sb")
            nc.scalar.activation(hsb, hps, AF.Gelu, bias=b1f[:, e, ft:ft + 1])
            h_tiles.append(hsb)
        for og in range(2):
            ops = [mps_o.tile([128, T2], F32, tag="o", name=f"o_{e}_{og}_{i}")
                   for i in range(4)]
            for ft in range(FT):
                w2t = mw2.tile([128, 512], F32R, tag="w2sl")
                nc.sync.dma_start(out=w2t, in_=dt["w2"][e, og, ft])
                for ot in range(4):
                    nc.tensor.matmul(ops[ot], w2t[:, ot * 128:(ot + 1) * 128],
                                     h_tiles[ft],
                                     start=(ft == 0), stop=(ft == FT - 1))
            for ot in range(4):
                oc = og * 4 + ot
                tmp = mwork.tile([128, T2], F32, tag="otmp")
                nc.vector.scalar_tensor_tensor(tmp, in0=ops[ot],
                                               scalar=b2f[:, e, oc:oc + 1],
                                               in1=probs_bc[:, e, :],
                                               op0=ALU.add, op1=ALU.mult)
                nc.vector.tensor_add(acc[:, oc, :], acc[:, oc, :], tmp)

    # ---- outputs: transpose back to token-major and store ---------------
    for s, oname in ((0, "oq"), (1, "oi")):
        for tt in range(2):
            otile = mwork.tile([128, H], F32, tag="otm")
            for oc in range(IC):
                tp = mps_h.tile([128, 128], F32, tag="tp")
                nc.tensor.transpose(
                    tp, acc[:, oc, s * T + tt * 128: s * T + (tt + 1) * 128], ident)
                nc.vector.tensor_copy(otile[:, oc * 128:(oc + 1) * 128], tp)
            nc.sync.dma_start(out=dt[oname][:, tt, :], in_=otile)

    for p in (mwork, mw2, mw1, hpool, mps_o, mps_h, dram, persist):
        p.release()


# ----------------------------------------------------------------------------
# host-side prep + run
# ----------------------------------------------------------------------------

_NC = None
LAST_EXEC_NS = None


def _get_nc():
    global _NC
    if _NC is None:
        _NC = _build_program()
    return _NC


def _prep_inputs(inp):
    """Build the per-core in_maps from the full (unsharded) numpy inputs."""
    f = np.float32

    def c(a):
        return np.ascontiguousarray(a, dtype=f)

    shared = {}
    shared["w_sa"] = c(inp["sa_in_w"].reshape(3, 8, 128, IC, 128).transpose(0, 1, 4, 3, 2))
    shared["b_sa"] = c(inp["sa_in_b"].reshape(3, IC, 128).transpose(2, 0, 1))
    shared["w_sao"] = c(inp["sa_out_w"].reshape(8, 128, IC, 128).transpose(0, 3, 2, 1))
    shared["b_sao"] = c(inp["sa_out_b"].reshape(IC, 128).T)
    shared["w_ca"] = c(inp["ca_in_w"].reshape(3, 8, 128, IC, 128).transpose(0, 1, 4, 3, 2))
    shared["b_ca"] = c(inp["ca_in_b"].reshape(3, IC, 128).transpose(2, 0, 1))
    shared["w_cao"] = c(inp["ca_out_w"].reshape(8, 128, IC, 128).transpose(0, 3, 2, 1))
    shared["b_cao"] = c(inp["ca_out_b"].reshape(IC, 128).T)
    shared["w_ig1"] = c(inp["img_gate_w"][:, :H].T.reshape(IC, 128, E).transpose(1, 0, 2))
    shared["w_ig2"] = c(inp["img_gate_w"][:, H:].T.reshape(IC, 128, E).transpose(1, 0, 2))
    shared["b_ig"] = c(inp["img_gate_b"][None, :])
    shared["w_tg1"] = c(inp["txt_gate_w"][:, :H].T.reshape(IC, 128, E).transpose(1, 0, 2))
    shared["w_tg2"] = c(inp["txt_gate_w"][:, H:].T.reshape(IC, 128, E).transpose(1, 0, 2))
    shared["b_tg"] = c(inp["txt_gate_b"][None, :])
    for n, k in (("g_lnq", "lnq_g"), ("b_lnq", "lnq_b"), ("g_lnc", "lnc_g"),
                 ("b_lnc", "lnc_b"), ("g_lnf", "lnf_g"), ("b_lnf", "lnf_b")):
        shared[n] = c(inp[k].reshape(IC, 128).T)
    shared["w1"] = c(inp["ew1"].reshape(E, IC, 128, FT, 128).transpose(0, 3, 2, 1, 4))
    shared["b1"] = c(inp["eb1"].reshape(E, FT, 128).transpose(2, 0, 1))
    shared["w2"] = c(inp["ew2"].reshape(E, FT, 128, 2, 512).transpose(0, 3, 1, 2, 4))
    shared["b2"] = c(inp["eb2"].reshape(E, IC, 128).transpose(2, 0, 1))

    def fm(a):  # [T, H] -> [128, IC, T]
        return c(a.T.reshape(IC, 128, T).transpose(1, 0, 2))

    in_maps = []
    for b in range(B):
        m = dict(shared)
        m["xq"] = fm(np.asarray(inp["query_tokens"][b]))
        m["xi"] = fm(np.asarray(inp["image_tokens"][b]))
        m["xt"] = fm(np.asarray(inp["text_context"][b]))
        in_maps.append(m)
    return in_maps


def _run(inp, trace=False):
    global LAST_EXEC_NS
    nc = _get_nc()
    in_maps = _prep_inputs(inp)
    res = run_bass_kernel_spmd(nc, in_maps, core_ids=list(range(B)), trace=trace)
    LAST_EXEC_NS = res.exec_time_ns
    oq = np.empty((B, T, H), np.float32)
    oi = np.empty((B, T, H), np.float32)
    for b in range(B):
        oq[b] = res.results[b]["oq"].transpose(1, 0, 2).reshape(T, H)
        oi[b] = res.results[b]["oi"].transpose(1, 0, 2).reshape(T, H)
    return oq, oi


def kernel(**inputs):
    return _run(inputs, trace=False)



# revision 10
# speedup vs baseline: 1.8066x; 1.8066x over previous
"""CrossModalMoELayer Trainium2 Bass kernel.

Sharding: data-parallel over batch B=8 across the 8 NeuronCores (one batch
element per core). Each core runs the full layer for its batch element:
self-attention, cross-attention, gating, and the dense 8-expert MoE
(weights replicated, streamed from HBM).

Layouts on device:
  feature-major ("fm"): [feat_part=128, feat_chunk, tokens]  - activations
  All matmuls run as float32r (full-rate fp32 PE mode, fp32 PSUM accumulate).

kernel(**inputs) takes the FULL unsharded inputs (numpy, keyed as in
setup_inputs()) and returns the full (query_tokens, image_tokens) tuple.
"""

import ml_dtypes
import numpy as np

import concourse.bass as bass
import concourse.tile as tile
from concourse import bacc, mybir
from concourse.bass_utils import run_bass_kernel_spmd
from concourse.masks import make_identity

B, T, H, NH, HD, F, E = 8, 256, 1024, 16, 64, 4096, 8
IC = H // 128          # 8 feature chunks of the model dim
FT = F // 128          # 32 feature chunks of the FFN dim
T2 = 2 * T             # query tokens + image tokens concatenated
EPS = 1e-5

F32 = mybir.dt.float32
F32R = mybir.dt.float32r
FP8 = mybir.dt.float8e4
PMDR = mybir.MatmulPerfMode.DoubleRow
AX = mybir.AxisListType
ALU = mybir.AluOpType
AF = mybir.ActivationFunctionType

# fp8 quantization scales: x8 = fp8(SX * x), w = fp8(SW * w_fp32)
SX = 8.0
SW = 512.0


def _r(ap):
    return ap.bitcast(F32R)


# ----------------------------------------------------------------------------
# program builder
# ----------------------------------------------------------------------------

def _build_program():
    nc = bacc.Bacc(
        "TRN2",
        target_bir_lowering=False,
        debug=False,
        enable_asserts=False,
        num_devices=8,
    )

    dt = {}

    def din(name, shape, d=F32):
        dt[name] = nc.dram_tensor(name, list(shape), d, kind="ExternalInput").ap()
        return dt[name]

    def dout(name, shape):
        dt[name] = nc.dram_tensor(name, list(shape), F32, kind="ExternalOutput").ap()
        return dt[name]

    # activations (per core)
    din("xq", (128, IC, T), F32R)
    din("xi", (128, IC, T), F32R)
    din("xt", (128, IC, T), F32R)
    # attention weights: [proj, ot, i(128), ic, o(128)]
    din("w_sa", (3, 8, 128, IC, 128), F32R)
    din("b_sa", (128, 3, IC))
    din("w_sao", (8, 128, IC, 128), F32R)
    din("b_sao", (128, IC))
    din("w_ca", (3, 8, 128, IC, 128), F32R)
    din("b_ca", (128, 3, IC))
    din("w_cao", (8, 128, IC, 128), F32R)
    din("b_cao", (128, IC))
    # gates
    din("w_ig1", (128, IC, E), F32R)
    din("w_ig2", (128, IC, E), F32R)
    din("b_ig", (1, E))
    din("w_tg1", (128, IC, E), F32R)
    din("w_tg2", (128, IC, E), F32R)
    din("b_tg", (1, E))
    # layernorms [128, IC]
    for n in ("g_lnq", "b_lnq", "g_lnc", "b_lnc", "g_lnf", "b_lnf"):
        din(n, (128, IC))
    # experts (fp8, pre-scaled by SW on host)
    din("w1", (E, FT, 128, IC, 128), FP8)      # [e, ft, i, ic, f]
    din("b1", (128, E, FT))
    din("w2", (E, IC, 128, FT, 128), FP8)      # [e, oc, i(f%128), ft, o]
    din("b2", (128, E, IC))
    # outputs [p(=t within tile), tt, o]
    dout("oq", (128, 2, H))
    dout("oi", (128, 2, H))

    with tile.TileContext(nc) as tc:
        _trace_kernel(nc, tc, dt)

    nc.compile()
    return nc


def _trace_kernel(nc, tc, dt):
    persist = tc.alloc_tile_pool(name="persist", bufs=1)

    # ---- constants + small params --------------------------------------
    ident = persist.tile([128, 128], F32, tag="ident")
    make_identity(nc, ident)
    ones_f = persist.tile([128, 1], F32, tag="ones_f")
    nc.vector.memset(ones_f, 1.0)
    ones = persist.tile([128, 1], F32R, tag="ones")
    nc.vector.tensor_copy(ones, ones_f)
    identr = persist.tile([128, 128], F32R, tag="identr")
    nc.vector.tensor_copy(identr, ident)
    eps_t = persist.tile([1, 1], F32, tag="eps")
    nc.vector.memset(eps_t, EPS)

    def load(name, shape, d=F32, pool=persist):
        t = pool.tile(list(shape), d, tag=f"ld_{name}")
        nc.sync.dma_start(out=t, in_=dt[name])
        return t

    xi0 = load("xi", (128, IC, T), F32R)
    xt0 = load("xt", (128, IC, T), F32R)
    b_sa = load("b_sa", (128, 3, IC))
    b_sao = load("b_sao", (128, IC))
    b_ca = load("b_ca", (128, 3, IC))
    b_cao = load("b_cao", (128, IC))
    w_ig1 = load("w_ig1", (128, IC, E), F32R)
    w_ig2 = load("w_ig2", (128, IC, E), F32R)
    b_ig = load("b_ig", (1, E))
    w_tg1 = load("w_tg1", (128, IC, E), F32R)
    w_tg2 = load("w_tg2", (128, IC, E), F32R)
    b_tg = load("b_tg", (1, E))
    lnp = {n: load(n, (128, IC)) for n in
           ("g_lnq", "b_lnq", "g_lnc", "b_lnc", "g_lnf", "b_lnf")}
    b1f = load("b1", (128, E, FT))
    b2f = load("b2", (128, E, IC))

    # persistent activations
    q2 = persist.tile([128, IC, T], F32R, tag="q2")          # query after CA
    x8 = persist.tile([128, IC, T2], FP8, tag="x8")          # SX*[lnf(q2) ; xi0]
    probs_bc = persist.tile([128, E, T2], F32, tag="probs") # router probs bcast
    acc = persist.tile([128, IC, T2], F32, tag="acc")       # MoE accumulator

    dram = tc.alloc_tile_pool(name="dram", bufs=1, space="DRAM")
    scr_probs = dram.tile([2, E, T], F32, tag="scr_probs")

    # ====================================================================
    # phase 1: attention + gating + lnf (own pools, released before MoE)
    # ====================================================================
    aps_mm = tc.alloc_tile_pool(name="aps_mm", bufs=3, space="PSUM")
    aps_tr = tc.alloc_tile_pool(name="aps_tr", bufs=2, space="PSUM")
    aps_pv = tc.alloc_tile_pool(name="aps_pv", bufs=2, space="PSUM")
    aps_sm = tc.alloc_tile_pool(name="aps_sm", bufs=1, space="PSUM")
    awork = tc.alloc_tile_pool(name="awork", bufs=2)
    aw1 = tc.alloc_tile_pool(name="aw1", bufs=1)
    wpool = tc.alloc_tile_pool(name="wpool", bufs=4)

    def ln_fm(dst, src, g, b):
        """dst[:, ic, :] = LN over features of src (fm layout [128, IC, T])."""
        ntok = src.shape[2]
        sum_ps = aps_sm.tile([1, ntok], F32, tag="sm")
        for ic in range(IC):
            nc.tensor.matmul(sum_ps, ones, src[:, ic, :],
                             start=(ic == 0), stop=(ic == IC - 1))
        mean = awork.tile([1, ntok], F32, tag="ln_mean")
        nc.scalar.mul(mean, sum_ps, 1.0 / H)
        sumsq_ps = aps_sm.tile([1, ntok], F32, tag="sm")
        for ic in range(IC):
            xsq = awork.tile([128, ntok], F32R, tag="ln_xsq")
            nc.scalar.activation(xsq, src[:, ic, :], AF.Square)
            nc.tensor.matmul(sumsq_ps, ones, xsq,
                             start=(ic == 0), stop=(ic == IC - 1))
        msq = awork.tile([1, ntok], F32, tag="ln_msq")
        nc.vector.tensor_mul(msq, mean, mean)
        var = awork.tile([1, ntok], F32, tag="ln_var")
        nc.vector.scalar_tensor_tensor(var, in0=sumsq_ps, scalar=1.0 / H,
                                       in1=msq, op0=ALU.mult, op1=ALU.subtract)
        std = awork.tile([1, ntok], F32, tag="ln_std")
        nc.scalar.activation(std, var, AF.Sqrt, bias=eps_t)
        rstd = awork.tile([1, ntok], F32, tag="ln_rstd")
        nc.vector.reciprocal(rstd, std)
        negc = awork.tile([1, ntok], F32, tag="ln_negc")
        nc.vector.scalar_tensor_tensor(negc, in0=mean, scalar=-1.0,
                                       in1=rstd, op0=ALU.mult, op1=ALU.mult)
        a_bc = awork.tile([128, ntok], F32, tag="ln_abc")
        nc.gpsimd.partition_broadcast(a_bc, rstd)
        c_bc = awork.tile([128, ntok], F32, tag="ln_cbc")
        nc.gpsimd.partition_broadcast(c_bc, negc)
        for ic in range(IC):
            tmp = awork.tile([128, ntok], F32, tag="ln_tmp")
            nc.vector.tensor_mul(tmp, src[:, ic, :], a_bc)
            nc.vector.tensor_add(tmp, tmp, c_bc)
            nc.vector.tensor_scalar(out=dst[:, ic, :], in0=tmp,
                                    scalar1=g[:, ic:ic + 1], scalar2=b[:, ic:ic + 1],
                                    op0=ALU.mult, op1=ALU.add)

    def proj_fm(dst, src, w_dram_ot, bias, bias_col):
        """dst[:, ot, :] = W @ src + b  (fm in, fm out).

        w_dram_ot(ot) -> DRAM AP [128, IC, 128]; bias[:, bias_col(ot)] is the
        per-partition bias column [128, 1].
        """
        ntok = src.shape[2]
        for ot in range(IC):
            wt = wpool.tile([128, IC, 128], F32R, tag="wsl")
            nc.sync.dma_start(out=wt, in_=w_dram_ot(ot))
            ps = aps_mm.tile([128, ntok], F32, tag="mm")
            for ic in range(IC):
                nc.tensor.matmul(ps, wt[:, ic, :], src[:, ic, :],
                                 start=(ic == 0), stop=(ic == IC - 1))
            nc.scalar.add(dst[:, ot, :], ps, bias[:, bias_col(ot)])

    def attention(new_resid, old_resid, qsrc, kvsrc, w_in, b_in, w_out, b_out):
        """new_resid = old_resid + out_proj(MHA(q=qsrc, kv=kvsrc)); all fm."""
        qf = aw1.tile([128, IC, T], F32R, tag="qf")
        kf = aw1.tile([128, IC, T], F32R, tag="kf")
        vf = aw1.tile([128, IC, T], F32R, tag="vf")
        proj_fm(qf, qsrc, lambda ot: w_in[0, ot], b_in, lambda ot: slice(0 * IC + ot, 0 * IC + ot + 1))
        proj_fm(kf, kvsrc, lambda ot: w_in[1, ot], b_in, lambda ot: slice(1 * IC + ot, 1 * IC + ot + 1))
        proj_fm(vf, kvsrc, lambda ot: w_in[2, ot], b_in, lambda ot: slice(2 * IC + ot, 2 * IC + ot + 1))
        # attention output, token-major: ao_tm[t(128), qt, h*64+d]
        ao_tm = aw1.tile([128, 2, H], F32R, tag="ao_tm")
        for pair in range(NH // 2):
            per_head = []
            for h in (2 * pair, 2 * pair + 1):
                base = (h % 2) * HD
                c = h // 2
                qh = qf[base:base + HD, c, :]
                kh = kf[base:base + HD, c, :]
                vh = vf[base:base + HD, c, :]
                idn = identr[base:base + HD, base:base + HD]
                # vh^T : [T, HD] in two 128-token tiles
                vht = awork.tile([128, 2, HD], F32R, tag="vht",
                                 name=f"vht_{h}")
                for kt in range(2):
                    tp = aps_tr.tile([128, HD], F32R, tag="tr")
                    nc.tensor.transpose(tp, vh[:, kt * 128:(kt + 1) * 128], idn)
                    nc.vector.tensor_copy(vht[:, kt, :], tp)
                attn_t = awork.tile([128, 2, T], F32R, tag="attnT",
                                    name=f"attnT_{h}")
                for qt in range(2):
                    sc = aps_mm.tile([128, T], F32, tag="mm")
                    nc.tensor.matmul(sc, qh[:, qt * 128:(qt + 1) * 128], kh,
                                     start=True, stop=True)
                    nmax = awork.tile([128, 1], F32, tag="nmax")
                    nc.vector.reduce_max(nmax, sc, axis=AX.X, negate=True)
                    nmax2 = awork.tile([128, 1], F32, tag="nmax2")
                    nc.scalar.mul(nmax2, nmax, 0.125)
                    asb = awork.tile([128, T], F32, tag="asb")
                    ssum = awork.tile([128, 1], F32, tag="ssum")
                    nc.scalar.activation(asb, sc, AF.Exp, bias=nmax2, scale=0.125,
                                         accum_out=ssum)
                    rsum = awork.tile([128, 1], F32, tag="rsum")
                    nc.vector.reciprocal(rsum, ssum)
                    asb_r = awork.tile([128, T], F32R, tag="asb_r")
                    nc.vector.tensor_scalar_mul(asb_r, asb, rsum)
                    for kt in range(2):
                        tp2 = aps_tr.tile([128, 128], F32R, tag="tr")
                        nc.tensor.transpose(tp2, asb_r[:, kt * 128:(kt + 1) * 128],
                                            identr)
                        nc.vector.tensor_copy(
                            attn_t[:, kt, qt * 128:(qt + 1) * 128], tp2)
                per_head.append((vht, attn_t))
            # PV for the pair, token-major: out[q, d] per qt into one psum tile
            for qt in range(2):
                pvp = aps_pv.tile([128, 2 * HD], F32, tag="pv")
                for j, (vht, attn_t) in enumerate(per_head):
                    for kt in range(2):
                        nc.tensor.matmul(pvp[:, j * HD:(j + 1) * HD],
                                         attn_t[:, kt, qt * 128:(qt + 1) * 128],
                                         vht[:, kt, :],
                                         start=(kt == 0), stop=(kt == 1))
                nc.scalar.copy(ao_tm[:, qt, pair * 2 * HD:(pair + 1) * 2 * HD], pvp)
        # transpose ao back to feature-major for the output projection
        ao = aw1.tile([128, IC, T], F32R, tag="ao")
        for oc in range(IC):
            for qt in range(2):
                tpo = aps_tr.tile([128, 128], F32R, tag="tr")
                nc.tensor.transpose(tpo, ao_tm[:, qt, oc * 128:(oc + 1) * 128],
                                    identr)
                nc.vector.tensor_copy(ao[:, oc, qt * 128:(qt + 1) * 128], tpo)
        # out-proj + bias + residual
        for ot in range(IC):
            wt = wpool.tile([128, IC, 128], F32R, tag="wsl")
            nc.sync.dma_start(out=wt, in_=w_out[ot])
            ps = aps_mm.tile([128, T], F32, tag="mm")
            for ic in range(IC):
                nc.tensor.matmul(ps, wt[:, ic, :], ao[:, ic, :],
                                 start=(ic == 0), stop=(ic == IC - 1))
            nc.vector.scalar_tensor_tensor(new_resid[:, ot, :], in0=ps,
                                           scalar=b_out[:, ot:ot + 1],
                                           in1=old_resid[:, ot, :],
                                           op0=ALU.add, op1=ALU.add)

    def gate(s, tokens_fm, w1sb, w2sb, bsb, ctx):
        """probs_bc[:, :, s*T:(s+1)*T] = softmax_E(tokens.W1 + ctx.W2 + b)."""
        ct_ps = aps_sm.tile([1, E], F32, tag="sm")
        for ic in range(IC):
            nc.tensor.matmul(ct_ps, ctx[:, ic, :], w2sb[:, ic, :],
                             start=(ic == 0), stop=(ic == IC - 1))
        crow = awork.tile([1, E], F32, tag="crow")
        nc.vector.tensor_add(crow, ct_ps, bsb)
        crow_bc = awork.tile([128, E], F32, tag="crow_bc")
        nc.gpsimd.partition_broadcast(crow_bc, crow)
        ptm = awork.tile([128, 2, E], F32, tag="ptm")
        for tt in range(2):
            lg_ps = aps_tr.tile([128, E], F32, tag="tr")
            for ic in range(IC):
                nc.tensor.matmul(lg_ps, tokens_fm[:, ic, tt * 128:(tt + 1) * 128],
                                 w1sb[:, ic, :],
                                 start=(ic == 0), stop=(ic == IC - 1))
            lg = awork.tile([128, E], F32, tag="lg")
            nc.vector.tensor_add(lg, lg_ps, crow_bc)
            nm = awork.tile([128, 1], F32, tag="gnm")
            nc.vector.reduce_max(nm, lg, axis=AX.X, negate=True)
            gs = awork.tile([128, 1], F32, tag="gs")
            nc.scalar.activation(ptm[:, tt, :], lg, AF.Exp, bias=nm, accum_out=gs)
            gr = awork.tile([128, 1], F32, tag="gr")
            nc.vector.reciprocal(gr, gs)
            nc.vector.tensor_scalar_mul(ptm[:, tt, :], ptm[:, tt, :], gr)
        # fold the 1/SW fp8 descale of the expert outputs into the probs
        pfm = awork.tile([E, 2, 128], F32, tag="pfm")
        for tt in range(2):
            tp = aps_tr.tile([E, 128], F32, tag="tr")
            nc.tensor.transpose(tp, ptm[:, tt, :], ident)
            nc.scalar.mul(pfm[:, tt, :], tp, 1.0 / SW)
        nc.sync.dma_start(out=scr_probs[s], in_=pfm)
        nc.sync.dma_start(out=probs_bc[:, :, s * T:(s + 1) * T],
                          in_=scr_probs[s].partition_broadcast(128))

    # ---- phase-1 body ---------------------------------------------------
    xq0 = aw1.tile([128, IC, T], F32R, tag="xq0")
    nc.sync.dma_start(out=xq0, in_=dt["xq"])

    qn = aw1.tile([128, IC, T], F32R, tag="qn")
    ln_fm(qn, xq0, lnp["g_lnq"], lnp["b_lnq"])
    q1 = aw1.tile([128, IC, T], F32R, tag="q1")
    attention(q1, xq0, qn, qn, dt["w_sa"], b_sa.rearrange("p a b -> p (a b)"),
              dt["w_sao"], b_sao)

    qn2 = aw1.tile([128, IC, T], F32R, tag="qn2")
    ln_fm(qn2, q1, lnp["g_lnc"], lnp["b_lnc"])
    attention(q2, q1, qn2, xi0, dt["w_ca"], b_ca.rearrange("p a b -> p (a b)"),
              dt["w_cao"], b_cao)

    # contexts: mean over tokens
    ictx = awork.tile([128, IC, 1], F32R, tag="ictx")
    tctx = awork.tile([128, IC, 1], F32R, tag="tctx")
    with nc.allow_low_precision(reason="f32r shares f32 bits; DVE sum is fp32"):
        for ic in range(IC):
            nc.vector.reduce_sum(ictx[:, ic, :], xi0[:, ic, :], axis=AX.X)
            nc.vector.reduce_sum(tctx[:, ic, :], xt0[:, ic, :], axis=AX.X)
    nc.scalar.mul(ictx.rearrange("p a b -> p (a b)"),
                  ictx.rearrange("p a b -> p (a b)"), 1.0 / T)
    nc.scalar.mul(tctx.rearrange("p a b -> p (a b)"),
                  tctx.rearrange("p a b -> p (a b)"), 1.0 / T)

    # routers: query stream uses txt gate on q2; image stream uses img gate
    gate(0, q2, w_tg1, w_tg2, b_tg, ictx)
    gate(1, xi0, w_ig1, w_ig2, b_ig, tctx)

    # moe input: SX*[ lnf(q2) ; xi0 ] quantized to fp8
    # (g_lnf/b_lnf are pre-scaled by SX on host, so ln_fm writes SX*lnf(q2))
    with nc.allow_low_precision(reason="fp8 moe input quantization"):
        ln_fm(x8[:, :, 0:T], q2, lnp["g_lnf"], lnp["b_lnf"])
        nc.scalar.mul(x8[:, :, T:T2], xi0, SX)

    # moe accumulator initialised with the residuals
    nc.vector.tensor_copy(acc[:, :, 0:T], q2)
    nc.vector.tensor_copy(acc[:, :, T:T2], xi0)

    for p in (wpool, aw1, awork, aps_sm, aps_pv, aps_tr, aps_mm):
        p.release()

    # ====================================================================
    # phase 2: dense MoE over both streams (512 tokens), fp8 DoubleRow
    # ====================================================================
    mps_h = tc.alloc_tile_pool(name="mps_h", bufs=2, space="PSUM")
    mps_o = tc.alloc_tile_pool(name="mps_o", bufs=2, space="PSUM")
    hpool = tc.alloc_tile_pool(name="hpool", bufs=2)
    mw1 = tc.alloc_tile_pool(name="mw1", bufs=4)
    mw2 = tc.alloc_tile_pool(name="mw2", bufs=2)
    mwork = tc.alloc_tile_pool(name="mwork", bufs=2)

    with nc.allow_low_precision(reason="fp8 moe"):
        for e in range(E):
            # hidden = fp8(GELU(x @ W1 + b1)); psum holds SX*SW*(x@W1)
            h_all = hpool.tile([128, FT, T2], FP8, tag="h_all")
            for ft in range(FT):
                w1t = mw1.tile([128, IC, 128], FP8, tag="w1sl")
                nc.sync.dma_start(out=w1t, in_=dt["w1"][e, ft])
                hps = mps_h.tile([128, T2], F32, tag="h")
                for c in range(IC // 2):
                    nc.tensor.matmul(hps, w1t[:, 2 * c:2 * c + 2, :],
                                     x8[:, 2 * c:2 * c + 2, :],
                                     start=(c == 0), stop=(c == IC // 2 - 1),
                                     perf_mode=PMDR)
                nc.scalar.activation(h_all[:, ft, :], hps, AF.Gelu,
                                     bias=b1f[:, e, ft:ft + 1],
                                     scale=1.0 / (SX * SW))
            # out = (h @ W2)*SW + SW*b2, mixed by probs/SW into acc
            for oc in range(IC):
                w2t = mw2.tile([128, FT, 128], FP8, tag="w2sl")
                nc.sync.dma_start(out=w2t, in_=dt["w2"][e, oc])
                ops = mps_o.tile([128, T2], F32, tag="o")
                for c in range(FT // 2):
                    nc.tensor.matmul(ops, w2t[:, 2 * c:2 * c + 2, :],
                                     h_all[:, 2 * c:2 * c + 2, :],
                                     start=(c == 0), stop=(c == FT // 2 - 1),
                                     perf_mode=PMDR)
                tmp = mwork.tile([128, T2], F32, tag="otmp")
                nc.vector.scalar_tensor_tensor(tmp, in0=ops,
                                               scalar=b2f[:, e, oc:oc + 1],
                                               in1=probs_bc[:, e, :],
                                               op0=ALU.add, op1=ALU.mult)
                nc.vector.tensor_add(acc[:, oc, :], acc[:, oc, :], tmp)

    # ---- outputs: transpose back to token-major and store ---------------
    for s, oname in ((0, "oq"), (1, "oi")):
        for tt in range(2):
            otile = mwork.tile([128, H], F32, tag="otm")
            for oc in range(IC):
                tp = mps_h.tile([128, 128], F32, tag="tp")
                nc.tensor.transpose(
                    tp, acc[:, oc, s * T + tt * 128: s * T + (tt + 1) * 128], ident)
                nc.vector.tensor_copy(otile[:, oc * 128:(oc + 1) * 128], tp)
            nc.sync.dma_start(out=dt[oname][:, tt, :], in_=otile)

    for p in (mwork, mw2, mw1, hpool, mps_o, mps_h, dram, persist):
        p.release()


# ----------------------------------------------------------------------------
# host-side prep + run
# ----------------------------------------------------------------------------

_NC = None
LAST_EXEC_NS = None


def _get_nc():
    global _NC
    if _NC is None:
        _NC = _build_program()
    return _NC


def _prep_inputs(inp):
    """Build the per-core in_maps from the full (unsharded) numpy inputs."""
    f = np.float32

    def c(a):
        return np.ascontiguousarray(a, dtype=f)

    shared = {}
    shared["w_sa"] = c(inp["sa_in_w"].reshape(3, 8, 128, IC, 128).transpose(0, 1, 4, 3, 2))
    shared["b_sa"] = c(inp["sa_in_b"].reshape(3, IC, 128).transpose(2, 0, 1))
    shared["w_sao"] = c(inp["sa_out_w"].reshape(8, 128, IC, 128).transpose(0, 3, 2, 1))
    shared["b_sao"] = c(inp["sa_out_b"].reshape(IC, 128).T)
    shared["w_ca"] = c(inp["ca_in_w"].reshape(3, 8, 128, IC, 128).transpose(0, 1, 4, 3, 2))
    shared["b_ca"] = c(inp["ca_in_b"].reshape(3, IC, 128).transpose(2, 0, 1))
    shared["w_cao"] = c(inp["ca_out_w"].reshape(8, 128, IC, 128).transpose(0, 3, 2, 1))
    shared["b_cao"] = c(inp["ca_out_b"].reshape(IC, 128).T)
    shared["w_ig1"] = c(inp["img_gate_w"][:, :H].T.reshape(IC, 128, E).transpose(1, 0, 2))
    shared["w_ig2"] = c(inp["img_gate_w"][:, H:].T.reshape(IC, 128, E).transpose(1, 0, 2))
    shared["b_ig"] = c(inp["img_gate_b"][None, :])
    shared["w_tg1"] = c(inp["txt_gate_w"][:, :H].T.reshape(IC, 128, E).transpose(1, 0, 2))
    shared["w_tg2"] = c(inp["txt_gate_w"][:, H:].T.reshape(IC, 128, E).transpose(1, 0, 2))
    shared["b_tg"] = c(inp["txt_gate_b"][None, :])
    for n, k in (("g_lnq", "lnq_g"), ("b_lnq", "lnq_b"), ("g_lnc", "lnc_g"),
                 ("b_lnc", "lnc_b"), ("g_lnf", "lnf_g"), ("b_lnf", "lnf_b")):
        shared[n] = c(inp[k].reshape(IC, 128).T)
    # lnf output is quantized to fp8 as SX*lnf(x): fold SX into gamma/beta
    shared["g_lnf"] = shared["g_lnf"] * np.float32(SX)
    shared["b_lnf"] = shared["b_lnf"] * np.float32(SX)

    def q8(a, s):
        return np.clip(np.asarray(a, np.float32) * np.float32(s),
                       -240.0, 240.0).astype(ml_dtypes.float8_e4m3)

    shared["w1"] = q8(np.asarray(inp["ew1"]).reshape(E, IC, 128, FT, 128)
                      .transpose(0, 3, 2, 1, 4), SW)
    shared["b1"] = c(inp["eb1"].reshape(E, FT, 128).transpose(2, 0, 1))
    shared["w2"] = q8(np.asarray(inp["ew2"]).reshape(E, FT, 128, IC, 128)
                      .transpose(0, 3, 2, 1, 4), SW)
    shared["b2"] = c(inp["eb2"].reshape(E, IC, 128).transpose(2, 0, 1)) * np.float32(SW)

    def fm(a):  # [T, H] -> [128, IC, T]
        return c(a.T.reshape(IC, 128, T).transpose(1, 0, 2))

    in_maps = []
    for b in range(B):
        m = dict(shared)
        m["xq"] = fm(np.asarray(inp["query_tokens"][b]))
        m["xi"] = fm(np.asarray(inp["image_tokens"][b]))
        m["xt"] = fm(np.asarray(inp["text_context"][b]))
        in_maps.append(m)
    return in_maps


def _run(inp, trace=False):
    global LAST_EXEC_NS
    nc = _get_nc()
    in_maps = _prep_inputs(inp)
    res = run_bass_kernel_spmd(nc, in_maps, core_ids=list(range(B)), trace=trace)
    LAST_EXEC_NS = res.exec_time_ns
    oq = np.empty((B, T, H), np.float32)
    oi = np.empty((B, T, H), np.float32)
    for b in range(B):
        oq[b] = res.results[b]["oq"].transpose(1, 0, 2).reshape(T, H)
        oi[b] = res.results[b]["oi"].transpose(1, 0, 2).reshape(T, H)
    return oq, oi


def kernel(**inputs):
    return _run(inputs, trace=False)



# revision 44
# speedup vs baseline: 1.9780x; 1.0949x over previous
"""CrossModalMoELayer Trainium2 Bass kernel.

Sharding: data-parallel over batch B=8 across the 8 NeuronCores (one batch
element per core). Each core runs the full layer for its batch element:
self-attention, cross-attention, gating, and the dense 8-expert MoE
(weights replicated, streamed from HBM).

Layouts on device:
  feature-major ("fm"): [feat_part=128, feat_chunk, tokens]  - activations
  All matmuls run as float32r (full-rate fp32 PE mode, fp32 PSUM accumulate).

kernel(**inputs) takes the FULL unsharded inputs (numpy, keyed as in
setup_inputs()) and returns the full (query_tokens, image_tokens) tuple.
"""

import ml_dtypes
import numpy as np

import concourse.bass as bass
import concourse.tile as tile
from concourse import bacc, mybir
from concourse.bass_utils import run_bass_kernel_spmd
from concourse.masks import make_identity

DEBUG = False
STAGE = 5
SKIP_LOADS = set()     # load names to skip (debug bisection)
SKIP_TILES = False     # skip q2/x8/probs/acc persist tiles (debug bisection)
B, T, H, NH, HD, F, E = 8, 256, 1024, 16, 64, 4096, 8
IC = H // 128          # 8 feature chunks of the model dim
FT = F // 128          # 32 feature chunks of the FFN dim
T2 = 2 * T             # query tokens + image tokens concatenated
EPS = 1e-5

F32 = mybir.dt.float32
F32R = mybir.dt.float32r
BF16 = mybir.dt.bfloat16
FP8 = mybir.dt.float8e4
PMDR = mybir.MatmulPerfMode.DoubleRow
AX = mybir.AxisListType
ALU = mybir.AluOpType
AF = mybir.ActivationFunctionType

# fp8 quantization scales: x8 = fp8(SX * x), w = fp8(SW * w_fp32)
SX = 8.0
SW = 512.0


def _r(ap):
    return ap.bitcast(F32R)


# ----------------------------------------------------------------------------
# program builder
# ----------------------------------------------------------------------------

def _build_program():
    nc = bacc.Bacc(
        "TRN2",
        target_bir_lowering=False,
        debug=False,
        enable_asserts=False,
        num_devices=8,
    )

    dt = {}

    def din(name, shape, d=F32):
        dt[name] = nc.dram_tensor(name, list(shape), d, kind="ExternalInput").ap()
        return dt[name]

    def dout(name, shape):
        dt[name] = nc.dram_tensor(name, list(shape), F32, kind="ExternalOutput").ap()
        return dt[name]

    # activations (per core)
    din("xq", (128, IC, T), F32R)
    din("xi", (128, IC, T), F32R)
    din("xt", (128, IC, T), F32R)
    # attention weights: [proj, ot, i(128), ic, o(128)] (bf16)
    din("w_sa", (3, 8, 128, IC, 128), BF16)
    din("b_sa", (128, 3, IC))
    din("w_sao", (8, 128, IC, 128), BF16)
    din("b_sao", (128, IC))
    din("w_ca", (3, 8, 128, IC, 128), BF16)
    din("b_ca", (128, 3, IC))
    din("w_cao", (8, 128, IC, 128), BF16)
    din("b_cao", (128, IC))
    # gates (biases packed in one 128-partition tensor: 1-partition DMAs
    # corrupt the low 16 bits of unrelated SBUF words on partitions 64-127)
    din("w_ig1", (128, IC, E), F32R)
    din("w_ig2", (128, IC, E), F32R)
    din("w_tg1", (128, IC, E), F32R)
    din("w_tg2", (128, IC, E), F32R)
    din("b_g", (128, 2, E))
    # layernorms [128, IC]
    for n in ("g_lnq", "b_lnq", "g_lnc", "b_lnc", "g_lnf", "b_lnf"):
        din(n, (128, IC))
    # experts (fp8, pre-scaled by SW on host)
    din("w1", (E, FT, 128, IC, 128), FP8)      # [e, ft, i, ic, f]
    din("b1", (128, E, FT))
    din("w2", (E, IC, 128, FT, 128), FP8)      # [e, oc, i(f%128), ft, o]
    din("b2", (128, E, IC))
    # outputs [p(=t within tile), tt, o]
    dout("oq", (128, 2, H))
    dout("oi", (128, 2, H))
    if DEBUG:
        dt["d_qn"] = nc.dram_tensor("d_qn", [128, IC, T], BF16, kind="ExternalOutput").ap()
        dt["d_qf"] = nc.dram_tensor("d_qf", [128, IC, T], BF16, kind="ExternalOutput").ap()
        dt["d_wt"] = nc.dram_tensor("d_wt", [128, IC, 128], BF16, kind="ExternalOutput").ap()
        dt["d_qnin"] = nc.dram_tensor("d_qnin", [128, IC, T], BF16, kind="ExternalOutput").ap()
        dt["d_ao"] = nc.dram_tensor("d_ao", [128, IC, T], BF16, kind="ExternalOutput").ap()
        dt["d_q1"] = nc.dram_tensor("d_q1", [128, IC, T], F32, kind="ExternalOutput").ap()
        dt["d_q2"] = nc.dram_tensor("d_q2", [128, IC, T], F32, kind="ExternalOutput").ap()
        dt["d_probs"] = nc.dram_tensor("d_probs", [128, E, T2], F32, kind="ExternalOutput").ap()
        dt["d_x8"] = nc.dram_tensor("d_x8", [128, IC, T2], FP8, kind="ExternalOutput").ap()

    with tile.TileContext(nc) as tc:
        _trace_kernel(nc, tc, dt)

    nc.compile()
    return nc


def _finish(nc, tc, dt, l):
    """Early-exit for STAGE bisection: dummy outputs + pool release."""
    out0 = l["persist"].tile([128, 2, H], F32, tag="dummy_out")
    nc.vector.memset(out0.rearrange("p a b -> p (a b)"), 0.0)
    nc.sync.dma_start(out=dt["oq"], in_=out0)
    nc.sync.dma_start(out=dt["oi"], in_=out0)
    for name in ("wpool", "aw1", "awork", "aps_sm", "aps_pv", "aps_tr",
                 "aps_mm", "dram", "persist"):
        if name in l:
            l[name].release()


def _trace_kernel(nc, tc, dt):
    persist = tc.alloc_tile_pool(name="persist", bufs=1)

    def load(name, shape, d=F32, pool=persist):
        t = pool.tile(list(shape), d, tag=f"ld_{name}")
        if name not in SKIP_LOADS:
            nc.sync.dma_start(out=t, in_=dt[name])
        else:
            ap = t if len(shape) <= 2 else t.rearrange("p a b -> p (a b)")
            nc.vector.memset(ap.bitcast(F32) if d == F32R else ap, 0.0)
        return t

    # critical-path activations first so phase-1 compute starts ASAP
    xq0 = persist.tile([128, IC, T], F32R, tag="xq0")
    nc.sync.dma_start(out=xq0, in_=dt["xq"])
    lnp = {n: load(n, (128, IC)) for n in
           ("g_lnq", "b_lnq", "g_lnc", "b_lnc", "g_lnf", "b_lnf")}
    xi0 = load("xi", (128, IC, T), F32R)
    xt0 = load("xt", (128, IC, T), F32R)

    # ---- constants + small params --------------------------------------
    ident = persist.tile([128, 128], F32, tag="ident")
    make_identity(nc, ident)
    ones_f = persist.tile([128, 1], F32, tag="ones_f")
    nc.vector.memset(ones_f, 1.0)
    ones = persist.tile([128, 1], F32R, tag="ones")
    nc.vector.tensor_copy(ones, ones_f)
    identr = persist.tile([128, 128], F32R, tag="identr")
    nc.vector.tensor_copy(identr, ident)
    eps_t = persist.tile([1, 1], F32, tag="eps")
    nc.vector.memset(eps_t, EPS)

    b_sa = load("b_sa", (128, 3, IC))
    b_sao = load("b_sao", (128, IC))
    b_ca = load("b_ca", (128, 3, IC))
    b_cao = load("b_cao", (128, IC))
    w_ig1 = load("w_ig1", (128, IC, E), F32R)
    w_ig2 = load("w_ig2", (128, IC, E), F32R)
    w_tg1 = load("w_tg1", (128, IC, E), F32R)
    w_tg2 = load("w_tg2", (128, IC, E), F32R)
    b_g = load("b_g", (128, 2, E))
    b_ig = b_g[0:1, 0, :]
    b_tg = b_g[0:1, 1, :]
    b1f = load("b1", (128, E, FT))
    b2f = load("b2", (128, E, IC))

    # persistent activations
    if not SKIP_TILES:
        q2 = persist.tile([128, IC, T], F32R, tag="q2")          # query after CA
        x8 = persist.tile([128, IC, T2], FP8, tag="x8")          # SX*[lnf(q2) ; xi0]
        probs_bc = persist.tile([128, E, T2], F32, tag="probs") # router probs bcast
        acc = persist.tile([128, IC, T2], F32, tag="acc")       # MoE accumulator

        dram = tc.alloc_tile_pool(name="dram", bufs=1, space="DRAM")
        scr_probs = dram.tile([2, E, T], F32, tag="scr_probs")

    # ====================================================================
    # phase 1: attention + gating + lnf (own pools, released before MoE)
    # ====================================================================
    aps_mm = tc.alloc_tile_pool(name="aps_mm", bufs=3, space="PSUM")
    aps_tr = tc.alloc_tile_pool(name="aps_tr", bufs=2, space="PSUM")
    aps_pv = tc.alloc_tile_pool(name="aps_pv", bufs=2, space="PSUM")
    aps_sm = tc.alloc_tile_pool(name="aps_sm", bufs=1, space="PSUM")
    awork = tc.alloc_tile_pool(name="awork", bufs=2)
    aw1 = tc.alloc_tile_pool(name="aw1", bufs=1)
    wpool = tc.alloc_tile_pool(name="wpool", bufs=6)

    def ln_fm(dst, src, g, b):
        """dst[:, ic, :] = LN over features of src (fm layout [128, IC, T])."""
        ntok = src.shape[2]
        sum_ps = aps_sm.tile([1, ntok], F32, tag="sm")
        for ic in range(IC):
            nc.tensor.matmul(sum_ps, ones, src[:, ic, :],
                             start=(ic == 0), stop=(ic == IC - 1))
        mean = awork.tile([1, ntok], F32, tag="ln_mean")
        nc.scalar.mul(mean, sum_ps, 1.0 / H)
        sumsq_ps = aps_sm.tile([1, ntok], F32, tag="sm")
        for ic in range(IC):
            xsq = awork.tile([128, ntok], F32R, tag="ln_xsq")
            nc.scalar.activation(xsq, src[:, ic, :], AF.Square)
            nc.tensor.matmul(sumsq_ps, ones, xsq,
                             start=(ic == 0), stop=(ic == IC - 1))
        msq = awork.tile([1, ntok], F32, tag="ln_msq")
        nc.vector.tensor_mul(msq, mean, mean)
        var = awork.tile([1, ntok], F32, tag="ln_var")
        nc.vector.scalar_tensor_tensor(var, in0=sumsq_ps, scalar=1.0 / H,
                                       in1=msq, op0=ALU.mult, op1=ALU.subtract)
        std = awork.tile([1, ntok], F32, tag="ln_std")
        nc.scalar.activation(std, var, AF.Sqrt, bias=eps_t)
        rstd = awork.tile([1, ntok], F32, tag="ln_rstd")
        nc.vector.reciprocal(rstd, std)
        negc = awork.tile([1, ntok], F32, tag="ln_negc")
        nc.vector.scalar_tensor_tensor(negc, in0=mean, scalar=-1.0,
                                       in1=rstd, op0=ALU.mult, op1=ALU.mult)
        a_bc = awork.tile([128, ntok], F32, tag="ln_abc")
        nc.gpsimd.partition_broadcast(a_bc, rstd)
        c_bc = awork.tile([128, ntok], F32, tag="ln_cbc")
        nc.gpsimd.partition_broadcast(c_bc, negc)
        for ic in range(IC):
            tmp = awork.tile([128, ntok], F32, tag="ln_tmp")
            nc.vector.tensor_mul(tmp, src[:, ic, :], a_bc)
            nc.vector.tensor_add(tmp, tmp, c_bc)
            nc.vector.tensor_scalar(out=dst[:, ic, :], in0=tmp,
                                    scalar1=g[:, ic:ic + 1], scalar2=b[:, ic:ic + 1],
                                    op0=ALU.mult, op1=ALU.add)

    def proj_fm(dst, src, w_dram_ot, bias, bias_col):
        """dst[:, ot, :] = W @ src + b  (fm in, fm out); src/W bf16."""
        ntok = src.shape[2]
        for ot in range(IC):
            wt = wpool.tile([128, IC, 128], BF16, tag="wsl")
            nc.sync.dma_start(out=wt, in_=w_dram_ot(ot))
            if DEBUG and dbg_first[0] and ot == 0:
                dbg_first[0] = False
                nc.sync.dma_start(out=dt["d_wt"], in_=wt)
                nc.sync.dma_start(out=dt["d_qnin"], in_=src)
            ps = aps_mm.tile([128, ntok], F32, tag="mm")
            for ic in range(IC):
                nc.tensor.matmul(ps, wt[:, ic, :], src[:, ic, :],
                                 start=(ic == 0), stop=(ic == IC - 1))
            nc.scalar.add(dst[:, ot, :], ps, bias[:, bias_col(ot)])

    dbg_first = [True]

    def attention(new_resid, old_resid, qsrc, kvsrc, w_in, b_in, w_out, b_out):
        """new_resid = old_resid + out_proj(MHA(q=qsrc, kv=kvsrc)); all fm.

        qsrc/kvsrc bf16; PV uses v^T as the stationary operand so the head
        outputs land directly feature-major (no post-transpose).
        """
        qf = aw1.tile([128, IC, T], BF16, tag="qf")
        kf = aw1.tile([128, IC, T], BF16, tag="kf")
        vf = aw1.tile([128, IC, T], F32R, tag="vf")
        proj_fm(qf, qsrc, lambda ot: w_in[0, ot], b_in, lambda ot: slice(0 * IC + ot, 0 * IC + ot + 1))
        proj_fm(kf, kvsrc, lambda ot: w_in[1, ot], b_in, lambda ot: slice(1 * IC + ot, 1 * IC + ot + 1))
        proj_fm(vf, kvsrc, lambda ot: w_in[2, ot], b_in, lambda ot: slice(2 * IC + ot, 2 * IC + ot + 1))
        # attention output, token-major: ao_tm[t(128), qt, h*64+d]
        ao_tm = aw1.tile([128, 2, H], F32R, tag="ao_tm")
        for pair in range(NH // 2):
            per_head = []
            for h in (2 * pair, 2 * pair + 1):
                base = (h % 2) * HD
                c = h // 2
                qh = qf[base:base + HD, c, :]
                kh = kf[base:base + HD, c, :]
                vh = vf[base:base + HD, c, :]
                idn = identr[base:base + HD, base:base + HD]
                # vh^T : [T, HD] in two 128-token tiles
                vht = awork.tile([128, 2, HD], F32R, tag="vht",
                                 name=f"vht_{h}")
                for kt in range(2):
                    tp = aps_tr.tile([128, HD], F32R, tag="tr")
                    nc.tensor.transpose(tp, vh[:, kt * 128:(kt + 1) * 128], idn)
                    nc.vector.tensor_copy(vht[:, kt, :], tp)
                attn_t = awork.tile([128, 2, T], F32R, tag="attnT",
                                    name=f"attnT_{h}")
                for qt in range(2):
                    sc = aps_mm.tile([128, T], F32, tag="mm")
                    nc.tensor.matmul(sc, qh[:, qt * 128:(qt + 1) * 128], kh,
                                     start=True, stop=True)
                    nmax = awork.tile([128, 1], F32, tag="nmax")
                    nc.vector.reduce_max(nmax, sc, axis=AX.X, negate=True)
                    nmax2 = awork.tile([128, 1], F32, tag="nmax2")
                    nc.scalar.mul(nmax2, nmax, 0.125)
                    asb = awork.tile([128, T], F32, tag="asb")
                    ssum = awork.tile([128, 1], F32, tag="ssum")
                    nc.scalar.activation(asb, sc, AF.Exp, bias=nmax2, scale=0.125,
                                         accum_out=ssum)
                    rsum = awork.tile([128, 1], F32, tag="rsum")
                    nc.vector.reciprocal(rsum, ssum)
                    asb_r = awork.tile([128, T], F32R, tag="asb_r")
                    nc.vector.tensor_scalar_mul(asb_r, asb, rsum)
                    for kt in range(2):
                        tp2 = aps_tr.tile([128, 128], F32R, tag="tr")
                        nc.tensor.transpose(tp2, asb_r[:, kt * 128:(kt + 1) * 128],
                                            identr)
                        nc.vector.tensor_copy(
                            attn_t[:, kt, qt * 128:(qt + 1) * 128], tp2)
                per_head.append((vht, attn_t))
            # PV for the pair, token-major: out[q, d] per qt into one psum tile
            for qt in range(2):
                pvp = aps_pv.tile([128, 2 * HD], F32, tag="pv")
                for j, (vht, attn_t) in enumerate(per_head):
                    for kt in range(2):
                        nc.tensor.matmul(pvp[:, j * HD:(j + 1) * HD],
                                         attn_t[:, kt, qt * 128:(qt + 1) * 128],
                                         vht[:, kt, :],
                                         start=(kt == 0), stop=(kt == 1))
                nc.scalar.copy(ao_tm[:, qt, pair * 2 * HD:(pair + 1) * 2 * HD], pvp)
        # transpose ao back to feature-major for the output projection
        ao = aw1.tile([128, IC, T], BF16, tag="ao")
        for oc in range(IC):
            for qt in range(2):
                tpo = aps_tr.tile([128, 128], F32R, tag="tr")
                nc.tensor.transpose(tpo, ao_tm[:, qt, oc * 128:(oc + 1) * 128],
                                    identr)
                nc.vector.tensor_copy(ao[:, oc, qt * 128:(qt + 1) * 128], tpo)
        if DEBUG and new_resid is not q2:
            nc.sync.dma_start(out=dt["d_qf"], in_=qf)
            nc.sync.dma_start(out=dt["d_ao"], in_=ao)
        # out-proj + bias + residual
        for ot in range(IC):
            wt = wpool.tile([128, IC, 128], BF16, tag="wsl")
            nc.sync.dma_start(out=wt, in_=w_out[ot])
            ps = aps_mm.tile([128, T], F32, tag="mm")
            for ic in range(IC):
                nc.tensor.matmul(ps, wt[:, ic, :], ao[:, ic, :],
                                 start=(ic == 0), stop=(ic == IC - 1))
            nc.vector.scalar_tensor_tensor(new_resid[:, ot, :], in0=ps,
                                           scalar=b_out[:, ot:ot + 1],
                                           in1=old_resid[:, ot, :],
                                           op0=ALU.add, op1=ALU.add)

    def gate(s, tokens_fm, w1sb, w2sb, bsb, ctx):
        """probs_bc[:, :, s*T:(s+1)*T] = softmax_E(tokens.W1 + ctx.W2 + b)."""
        ct_ps = aps_sm.tile([1, E], F32, tag="sm")
        for ic in range(IC):
            nc.tensor.matmul(ct_ps, ctx[:, ic, :], w2sb[:, ic, :],
                             start=(ic == 0), stop=(ic == IC - 1))
        crow = awork.tile([1, E], F32, tag="crow")
        nc.vector.tensor_add(crow, ct_ps, bsb)
        crow_bc = awork.tile([128, E], F32, tag="crow_bc")
        nc.gpsimd.partition_broadcast(crow_bc, crow)
        ptm = awork.tile([128, 2, E], F32, tag="ptm")
        for tt in range(2):
            lg_ps = aps_tr.tile([128, E], F32, tag="tr")
            for ic in range(IC):
                nc.tensor.matmul(lg_ps, tokens_fm[:, ic, tt * 128:(tt + 1) * 128],
                                 w1sb[:, ic, :],
                                 start=(ic == 0), stop=(ic == IC - 1))
            lg = awork.tile([128, E], F32, tag="lg")
            nc.vector.tensor_add(lg, lg_ps, crow_bc)
            nm = awork.tile([128, 1], F32, tag="gnm")
            nc.vector.reduce_max(nm, lg, axis=AX.X, negate=True)
            gs = awork.tile([128, 1], F32, tag="gs")
            nc.scalar.activation(ptm[:, tt, :], lg, AF.Exp, bias=nm, accum_out=gs)
            gr = awork.tile([128, 1], F32, tag="gr")
            nc.vector.reciprocal(gr, gs)
            nc.vector.tensor_scalar_mul(ptm[:, tt, :], ptm[:, tt, :], gr)
        # fold the 1/SW fp8 descale of the expert outputs into the probs
        pfm = awork.tile([E, 2, 128], F32, tag="pfm")
        for tt in range(2):
            tp = aps_tr.tile([E, 128], F32, tag="tr")
            nc.tensor.transpose(tp, ptm[:, tt, :], ident)
            nc.scalar.mul(pfm[:, tt, :], tp, 1.0 / SW)
        nc.sync.dma_start(out=scr_probs[s], in_=pfm)
        nc.sync.dma_start(out=probs_bc[:, :, s * T:(s + 1) * T],
                          in_=scr_probs[s].partition_broadcast(128))

    # ---- phase-1 body ---------------------------------------------------
    qn = aw1.tile([128, IC, T], BF16, tag="qn")
    with nc.allow_low_precision(reason="bf16 attention operands"):
        ln_fm(qn, xq0, lnp["g_lnq"], lnp["b_lnq"])

    if STAGE == 1:
        if DEBUG:
            nc.sync.dma_start(out=dt["d_qnin"], in_=qn)
        if not SKIP_TILES:
            nc.vector.memset(acc.rearrange("p a b -> p (a b)"), 0.0)
        _finish(nc, tc, dt, locals())
        return
    if STAGE == 0.5:
        # pure DVE cast copy into bf16 (no LN math)
        qc = aw1.tile([128, IC, T], BF16, tag="qc")
        with nc.allow_low_precision(reason="dbg"):
            nc.vector.tensor_copy(qc, xq0)
        # pure DMA import of bf16 weights
        wimp = aw1.tile([128, IC, 128], BF16, tag="wimp")
        nc.sync.dma_start(out=wimp, in_=dt["w_sa"][0, 0])
        if DEBUG:
            nc.sync.dma_start(out=dt["d_qnin"], in_=qc)
            nc.sync.dma_start(out=dt["d_wt"], in_=wimp)
        if not SKIP_TILES:
            nc.vector.memset(acc.rearrange("p a b -> p (a b)"), 0.0)
        _finish(nc, tc, dt, locals())
        return

    q1 = aw1.tile([128, IC, T], F32R, tag="q1")
    attention(q1, xq0, qn, qn, dt["w_sa"], b_sa.rearrange("p a b -> p (a b)"),
              dt["w_sao"], b_sao)
    if STAGE == 2:
        if DEBUG:
            nc.sync.dma_start(out=dt["d_q1"], in_=q1.bitcast(F32))
        if not SKIP_TILES:
            nc.vector.memset(acc.rearrange("p a b -> p (a b)"), 0.0)
        _finish(nc, tc, dt, locals())
        return

    xib = persist.tile([128, IC, T], BF16, tag="xib")
    with nc.allow_low_precision(reason="bf16 attention operands / fp8 moe"):
        nc.vector.tensor_copy(xib, xi0)
        nc.scalar.mul(x8[:, :, T:T2], xi0, SX)
    nc.vector.tensor_copy(acc[:, :, T:T2], xi0)

    qn2 = aw1.tile([128, IC, T], BF16, tag="qn2")
    with nc.allow_low_precision(reason="bf16 attention operands"):
        ln_fm(qn2, q1, lnp["g_lnc"], lnp["b_lnc"])
    attention(q2, q1, qn2, xib, dt["w_ca"], b_ca.rearrange("p a b -> p (a b)"),
              dt["w_cao"], b_cao)
    if STAGE == 3:
        if DEBUG:
            nc.sync.dma_start(out=dt["d_q2"], in_=q2.bitcast(F32))
        _finish(nc, tc, dt, locals())
        return

    ictx = awork.tile([128, IC, 1], F32R, tag="ictx")
    tctx = awork.tile([128, IC, 1], F32R, tag="tctx")
    with nc.allow_low_precision(reason="f32r shares f32 bits; DVE sum is fp32"):
        for ic in range(IC):
            nc.vector.reduce_sum(ictx[:, ic, :], xi0[:, ic, :], axis=AX.X)
            nc.vector.reduce_sum(tctx[:, ic, :], xt0[:, ic, :], axis=AX.X)
    nc.scalar.mul(ictx.rearrange("p a b -> p (a b)"),
                  ictx.rearrange("p a b -> p (a b)"), 1.0 / T)
    nc.scalar.mul(tctx.rearrange("p a b -> p (a b)"),
                  tctx.rearrange("p a b -> p (a b)"), 1.0 / T)
    gate(0, q2, w_tg1, w_tg2, b_tg, ictx)
    gate(1, xi0, w_ig1, w_ig2, b_ig, tctx)

    # moe input: SX*[ lnf(q2) ; xi0 ] quantized to fp8
    # (g_lnf/b_lnf are pre-scaled by SX on host, so ln_fm writes SX*lnf(q2))
    with nc.allow_low_precision(reason="fp8 moe input quantization"):
        ln_fm(x8[:, :, 0:T], q2, lnp["g_lnf"], lnp["b_lnf"])

    # moe accumulator initialised with the query residual
    nc.vector.tensor_copy(acc[:, :, 0:T], q2)

    if DEBUG:
        nc.sync.dma_start(out=dt["d_qn"], in_=qn)
        nc.sync.dma_start(out=dt["d_q1"], in_=q1.bitcast(F32))
        nc.sync.dma_start(out=dt["d_q2"], in_=q2.bitcast(F32))
        nc.sync.dma_start(out=dt["d_probs"], in_=probs_bc)
        nc.sync.dma_start(out=dt["d_x8"], in_=x8)

    for p in (wpool, aw1, awork, aps_sm, aps_pv, aps_tr, aps_mm):
        p.release()

    # ====================================================================
    # phase 2: dense MoE over both streams (512 tokens), fp8 DoubleRow
    # ====================================================================
    mps_h = tc.alloc_tile_pool(name="mps_h", bufs=2, space="PSUM")
    mps_o = tc.alloc_tile_pool(name="mps_o", bufs=2, space="PSUM")
    hpool = tc.alloc_tile_pool(name="hpool", bufs=2)
    mw1 = tc.alloc_tile_pool(name="mw1", bufs=4)
    mw2 = tc.alloc_tile_pool(name="mw2", bufs=2)
    mwork = tc.alloc_tile_pool(name="mwork", bufs=2)

    with nc.allow_low_precision(reason="fp8 moe"):
        for e in range(E):
            # hidden = fp8(GELU(x @ W1 + b1)); psum holds SX*SW*(x@W1)
            h_all = hpool.tile([128, FT, T2], FP8, tag="h_all")
            for ft in range(FT):
                w1t = mw1.tile([128, IC, 128], FP8, tag="w1sl")
                nc.sync.dma_start(out=w1t, in_=dt["w1"][e, ft])
                hps = mps_h.tile([128, T2], F32, tag="h")
                for c in range(IC // 2):
                    nc.tensor.matmul(hps, w1t[:, 2 * c:2 * c + 2, :],
                                     x8[:, 2 * c:2 * c + 2, :],
                                     start=(c == 0), stop=(c == IC // 2 - 1),
                                     perf_mode=PMDR)
                nc.scalar.activation(h_all[:, ft, :], hps, AF.Gelu,
                                     bias=b1f[:, e, ft:ft + 1],
                                     scale=1.0 / (SX * SW))
            # out = (h @ W2)*SW + SW*b2, mixed by probs/SW into acc
            for oc in range(IC):
                w2t = mw2.tile([128, FT, 128], FP8, tag="w2sl")
                nc.sync.dma_start(out=w2t, in_=dt["w2"][e, oc])
                ops = mps_o.tile([128, T2], F32, tag="o")
                for c in range(FT // 2):
                    nc.tensor.matmul(ops, w2t[:, 2 * c:2 * c + 2, :],
                                     h_all[:, 2 * c:2 * c + 2, :],
                                     start=(c == 0), stop=(c == FT // 2 - 1),
                                     perf_mode=PMDR)
                tmp = mwork.tile([128, T2], F32, tag="otmp")
                nc.vector.scalar_tensor_tensor(tmp, in0=ops,
                                               scalar=b2f[:, e, oc:oc + 1],
                                               in1=probs_bc[:, e, :],
                                               op0=ALU.add, op1=ALU.mult)
                nc.vector.tensor_add(acc[:, oc, :], acc[:, oc, :], tmp)

    # ---- outputs: transpose back to token-major and store ---------------
    for s, oname in ((0, "oq"), (1, "oi")):
        for tt in range(2):
            otile = mwork.tile([128, H], F32, tag="otm")
            for oc in range(IC):
                tp = mps_h.tile([128, 128], F32, tag="tp")
                nc.tensor.transpose(
                    tp, acc[:, oc, s * T + tt * 128: s * T + (tt + 1) * 128], ident)
                nc.vector.tensor_copy(otile[:, oc * 128:(oc + 1) * 128], tp)
            nc.sync.dma_start(out=dt[oname][:, tt, :], in_=otile)

    for p in (mwork, mw2, mw1, hpool, mps_o, mps_h, dram, persist):
        p.release()


# ----------------------------------------------------------------------------
# host-side prep + run
# ----------------------------------------------------------------------------

_NC = None
LAST_EXEC_NS = None


def _get_nc():
    global _NC
    if _NC is None:
        _NC = _build_program()
    return _NC


def _prep_inputs(inp):
    """Build the per-core in_maps from the full (unsharded) numpy inputs."""
    f = np.float32

    def c(a):
        return np.ascontiguousarray(a, dtype=f)

    def cb(a):  # bf16 attention weights
        return np.ascontiguousarray(a, dtype=f).astype(ml_dtypes.bfloat16)

    shared = {}
    shared["w_sa"] = cb(inp["sa_in_w"].reshape(3, 8, 128, IC, 128).transpose(0, 1, 4, 3, 2))
    shared["b_sa"] = c(inp["sa_in_b"].reshape(3, IC, 128).transpose(2, 0, 1))
    shared["w_sao"] = cb(inp["sa_out_w"].reshape(8, 128, IC, 128).transpose(0, 3, 2, 1))
    shared["b_sao"] = c(inp["sa_out_b"].reshape(IC, 128).T)
    shared["w_ca"] = cb(inp["ca_in_w"].reshape(3, 8, 128, IC, 128).transpose(0, 1, 4, 3, 2))
    shared["b_ca"] = c(inp["ca_in_b"].reshape(3, IC, 128).transpose(2, 0, 1))
    shared["w_cao"] = cb(inp["ca_out_w"].reshape(8, 128, IC, 128).transpose(0, 3, 2, 1))
    shared["b_cao"] = c(inp["ca_out_b"].reshape(IC, 128).T)
    shared["w_ig1"] = c(inp["img_gate_w"][:, :H].T.reshape(IC, 128, E).transpose(1, 0, 2))
    shared["w_ig2"] = c(inp["img_gate_w"][:, H:].T.reshape(IC, 128, E).transpose(1, 0, 2))
    shared["w_tg1"] = c(inp["txt_gate_w"][:, :H].T.reshape(IC, 128, E).transpose(1, 0, 2))
    shared["w_tg2"] = c(inp["txt_gate_w"][:, H:].T.reshape(IC, 128, E).transpose(1, 0, 2))
    shared["b_g"] = c(np.broadcast_to(
        np.stack([np.asarray(inp["img_gate_b"]), np.asarray(inp["txt_gate_b"])]),
        (128, 2, E)))
    for n, k in (("g_lnq", "lnq_g"), ("b_lnq", "lnq_b"), ("g_lnc", "lnc_g"),
                 ("b_lnc", "lnc_b"), ("g_lnf", "lnf_g"), ("b_lnf", "lnf_b")):
        shared[n] = c(inp[k].reshape(IC, 128).T)
    # lnf output is quantized to fp8 as SX*lnf(x): fold SX into gamma/beta
    shared["g_lnf"] = shared["g_lnf"] * np.float32(SX)
    shared["b_lnf"] = shared["b_lnf"] * np.float32(SX)

    def q8(a, s):
        return np.clip(np.asarray(a, np.float32) * np.float32(s),
                       -240.0, 240.0).astype(ml_dtypes.float8_e4m3)

    shared["w1"] = q8(np.asarray(inp["ew1"]).reshape(E, IC, 128, FT, 128)
                      .transpose(0, 3, 2, 1, 4), SW)
    shared["b1"] = c(inp["eb1"].reshape(E, FT, 128).transpose(2, 0, 1))
    shared["w2"] = q8(np.asarray(inp["ew2"]).reshape(E, FT, 128, IC, 128)
                      .transpose(0, 3, 2, 1, 4), SW)
    shared["b2"] = c(inp["eb2"].reshape(E, IC, 128).transpose(2, 0, 1)) * np.float32(SW)

    def fm(a):  # [T, H] -> [128, IC, T]
        return c(a.T.reshape(IC, 128, T).transpose(1, 0, 2))

    in_maps = []
    for b in range(B):
        m = dict(shared)
        m["xq"] = fm(np.asarray(inp["query_tokens"][b]))
        m["xi"] = fm(np.asarray(inp["image_tokens"][b]))
        m["xt"] = fm(np.asarray(inp["text_context"][b]))
        in_maps.append(m)
    return in_maps


def _run(inp, trace=False):
    global LAST_EXEC_NS
    nc = _get_nc()
    in_maps = _prep_inputs(inp)
    res = run_bass_kernel_spmd(nc, in_maps, core_ids=list(range(B)), trace=trace)
    LAST_EXEC_NS = res.exec_time_ns
    oq = np.empty((B, T, H), np.float32)
    oi = np.empty((B, T, H), np.float32)
    for b in range(B):
        oq[b] = res.results[b]["oq"].transpose(1, 0, 2).reshape(T, H)
        oi[b] = res.results[b]["oi"].transpose(1, 0, 2).reshape(T, H)
    return oq, oi


def kernel(**inputs):
    return _run(inputs, trace=False)



# revision 48
# speedup vs baseline: 2.0264x; 1.0245x over previous
"""CrossModalMoELayer Trainium2 Bass kernel.

Sharding: data-parallel over batch B=8 across the 8 NeuronCores (one batch
element per core). Each core runs the full layer for its batch element:
self-attention, cross-attention, gating, and the dense 8-expert MoE
(weights replicated, streamed from HBM).

Layouts on device:
  feature-major ("fm"): [feat_part=128, feat_chunk, tokens]  - activations
  All matmuls run as float32r (full-rate fp32 PE mode, fp32 PSUM accumulate).

kernel(**inputs) takes the FULL unsharded inputs (numpy, keyed as in
setup_inputs()) and returns the full (query_tokens, image_tokens) tuple.
"""

import ml_dtypes
import numpy as np

import concourse.bass as bass
import concourse.tile as tile
from concourse import bacc, mybir
from concourse.bass_utils import run_bass_kernel_spmd
from concourse.masks import make_identity

DEBUG = False
STAGE = 5
SKIP_LOADS = set()     # load names to skip (debug bisection)
SKIP_TILES = False     # skip q2/x8/probs/acc persist tiles (debug bisection)
B, T, H, NH, HD, F, E = 8, 256, 1024, 16, 64, 4096, 8
IC = H // 128          # 8 feature chunks of the model dim
FT = F // 128          # 32 feature chunks of the FFN dim
T2 = 2 * T             # query tokens + image tokens concatenated
EPS = 1e-5

F32 = mybir.dt.float32
F32R = mybir.dt.float32r
BF16 = mybir.dt.bfloat16
FP8 = mybir.dt.float8e4
PMDR = mybir.MatmulPerfMode.DoubleRow
AX = mybir.AxisListType
ALU = mybir.AluOpType
AF = mybir.ActivationFunctionType

# fp8 quantization scales: x8 = fp8(SX * x), w = fp8(SW * w_fp32)
SX = 8.0
SW = 512.0


def _r(ap):
    return ap.bitcast(F32R)


# ----------------------------------------------------------------------------
# program builder
# ----------------------------------------------------------------------------

def _build_program():
    nc = bacc.Bacc(
        "TRN2",
        target_bir_lowering=False,
        debug=False,
        enable_asserts=False,
        num_devices=8,
    )

    dt = {}

    def din(name, shape, d=F32):
        dt[name] = nc.dram_tensor(name, list(shape), d, kind="ExternalInput").ap()
        return dt[name]

    def dout(name, shape):
        dt[name] = nc.dram_tensor(name, list(shape), F32, kind="ExternalOutput").ap()
        return dt[name]

    # activations (per core)
    din("xq", (128, IC, T), F32R)
    din("xi", (128, IC, T), F32R)
    din("xt", (128, IC, T), F32R)
    # attention weights: [proj, ot, i(128), ic, o(128)] (bf16)
    din("w_sa", (3, 8, 128, IC, 128), BF16)
    din("b_sa", (128, 3, IC))
    din("w_sao", (8, 128, IC, 128), BF16)
    din("b_sao", (128, IC))
    din("w_ca", (3, 8, 128, IC, 128), BF16)
    din("b_ca", (128, 3, IC))
    din("w_cao", (8, 128, IC, 128), BF16)
    din("b_cao", (128, IC))
    # gates (biases packed in one 128-partition tensor: 1-partition DMAs
    # corrupt the low 16 bits of unrelated SBUF words on partitions 64-127)
    din("w_ig1", (128, IC, E), F32R)
    din("w_ig2", (128, IC, E), F32R)
    din("w_tg1", (128, IC, E), F32R)
    din("w_tg2", (128, IC, E), F32R)
    din("b_g", (128, 2, E))
    # layernorms [128, IC]
    for n in ("g_lnq", "b_lnq", "g_lnc", "b_lnc", "g_lnf", "b_lnf"):
        din(n, (128, IC))
    # experts (fp8, pre-scaled by SW on host)
    din("w1", (E, FT, 128, IC, 128), FP8)      # [e, ft, i, ic, f]
    din("b1", (128, E, FT))
    din("w2", (E, IC, 128, FT, 128), FP8)      # [e, oc, i(f%128), ft, o]
    din("b2", (128, E, IC))
    # outputs [p(=t within tile), tt, o]
    dout("oq", (128, 2, H))
    dout("oi", (128, 2, H))
    if DEBUG:
        dt["d_qn"] = nc.dram_tensor("d_qn", [128, IC, T], BF16, kind="ExternalOutput").ap()
        dt["d_qf"] = nc.dram_tensor("d_qf", [128, IC, T], BF16, kind="ExternalOutput").ap()
        dt["d_wt"] = nc.dram_tensor("d_wt", [128, IC, 128], BF16, kind="ExternalOutput").ap()
        dt["d_qnin"] = nc.dram_tensor("d_qnin", [128, IC, T], BF16, kind="ExternalOutput").ap()
        dt["d_ao"] = nc.dram_tensor("d_ao", [128, IC, T], BF16, kind="ExternalOutput").ap()
        dt["d_q1"] = nc.dram_tensor("d_q1", [128, IC, T], F32, kind="ExternalOutput").ap()
        dt["d_q2"] = nc.dram_tensor("d_q2", [128, IC, T], F32, kind="ExternalOutput").ap()
        dt["d_probs"] = nc.dram_tensor("d_probs", [128, E, T2], F32, kind="ExternalOutput").ap()
        dt["d_x8"] = nc.dram_tensor("d_x8", [128, IC, T2], FP8, kind="ExternalOutput").ap()

    with tile.TileContext(nc) as tc:
        _trace_kernel(nc, tc, dt)

    nc.compile()
    return nc


def _finish(nc, tc, dt, l):
    """Early-exit for STAGE bisection: dummy outputs + pool release."""
    out0 = l["persist"].tile([128, 2, H], F32, tag="dummy_out")
    nc.vector.memset(out0.rearrange("p a b -> p (a b)"), 0.0)
    nc.sync.dma_start(out=dt["oq"], in_=out0)
    nc.sync.dma_start(out=dt["oi"], in_=out0)
    for name in ("wpool", "aw1", "awork", "aps_sm", "aps_pv", "aps_tr",
                 "aps_mm", "dram", "persist"):
        if name in l:
            l[name].release()


def _trace_kernel(nc, tc, dt):
    persist = tc.alloc_tile_pool(name="persist", bufs=1)

    def load(name, shape, d=F32, pool=persist):
        t = pool.tile(list(shape), d, tag=f"ld_{name}")
        if name not in SKIP_LOADS:
            nc.sync.dma_start(out=t, in_=dt[name])
        else:
            ap = t if len(shape) <= 2 else t.rearrange("p a b -> p (a b)")
            nc.vector.memset(ap.bitcast(F32) if d == F32R else ap, 0.0)
        return t

    # critical-path activations first so phase-1 compute starts ASAP
    xq0 = persist.tile([128, IC, T], F32R, tag="xq0")
    nc.sync.dma_start(out=xq0, in_=dt["xq"])
    lnp = {n: load(n, (128, IC)) for n in
           ("g_lnq", "b_lnq", "g_lnc", "b_lnc", "g_lnf", "b_lnf")}
    xi0 = load("xi", (128, IC, T), F32R)
    xt0 = load("xt", (128, IC, T), F32R)

    # ---- constants + small params --------------------------------------
    ident = persist.tile([128, 128], F32, tag="ident")
    make_identity(nc, ident)
    ones_f = persist.tile([128, 1], F32, tag="ones_f")
    nc.vector.memset(ones_f, 1.0)
    ones = persist.tile([128, 1], F32R, tag="ones")
    nc.vector.tensor_copy(ones, ones_f)
    identb = persist.tile([128, 128], BF16, tag="identb")
    nc.vector.tensor_copy(identb, ident)
    eps_t = persist.tile([1, 1], F32, tag="eps")
    nc.vector.memset(eps_t, EPS)

    b_sa = load("b_sa", (128, 3, IC))
    b_sao = load("b_sao", (128, IC))
    b_ca = load("b_ca", (128, 3, IC))
    b_cao = load("b_cao", (128, IC))
    w_ig1 = load("w_ig1", (128, IC, E), F32R)
    w_ig2 = load("w_ig2", (128, IC, E), F32R)
    w_tg1 = load("w_tg1", (128, IC, E), F32R)
    w_tg2 = load("w_tg2", (128, IC, E), F32R)
    b_g = load("b_g", (128, 2, E))
    b_ig = b_g[0:1, 0, :]
    b_tg = b_g[0:1, 1, :]
    b1f = load("b1", (128, E, FT))
    b2f = load("b2", (128, E, IC))

    # persistent activations
    if not SKIP_TILES:
        q2 = persist.tile([128, IC, T], F32R, tag="q2")          # query after CA
        x8 = persist.tile([128, IC, T2], FP8, tag="x8")          # SX*[lnf(q2) ; xi0]
        probs_bc = persist.tile([128, E, T2], F32, tag="probs") # router probs bcast
        acc = persist.tile([128, IC, T2], F32, tag="acc")       # MoE accumulator

        dram = tc.alloc_tile_pool(name="dram", bufs=1, space="DRAM")
        scr_probs = dram.tile([2, E, T], F32, tag="scr_probs")

    # ====================================================================
    # phase 1: attention + gating + lnf (own pools, released before MoE)
    # ====================================================================
    aps_mm = tc.alloc_tile_pool(name="aps_mm", bufs=3, space="PSUM")
    aps_tr = tc.alloc_tile_pool(name="aps_tr", bufs=2, space="PSUM")
    aps_pv = tc.alloc_tile_pool(name="aps_pv", bufs=2, space="PSUM")
    aps_sm = tc.alloc_tile_pool(name="aps_sm", bufs=1, space="PSUM")
    awork = tc.alloc_tile_pool(name="awork", bufs=2)
    aw1 = tc.alloc_tile_pool(name="aw1", bufs=1)
    wpool = tc.alloc_tile_pool(name="wpool", bufs=6)

    def ln_fm(dst, src, g, b):
        """dst[:, ic, :] = LN over features of src (fm layout [128, IC, T])."""
        ntok = src.shape[2]
        sum_ps = aps_sm.tile([1, ntok], F32, tag="sm")
        for ic in range(IC):
            nc.tensor.matmul(sum_ps, ones, src[:, ic, :],
                             start=(ic == 0), stop=(ic == IC - 1))
        mean = awork.tile([1, ntok], F32, tag="ln_mean")
        nc.scalar.mul(mean, sum_ps, 1.0 / H)
        sumsq_ps = aps_sm.tile([1, ntok], F32, tag="sm")
        for ic in range(IC):
            xsq = awork.tile([128, ntok], F32R, tag="ln_xsq")
            nc.scalar.activation(xsq, src[:, ic, :], AF.Square)
            nc.tensor.matmul(sumsq_ps, ones, xsq,
                             start=(ic == 0), stop=(ic == IC - 1))
        msq = awork.tile([1, ntok], F32, tag="ln_msq")
        nc.vector.tensor_mul(msq, mean, mean)
        var = awork.tile([1, ntok], F32, tag="ln_var")
        nc.vector.scalar_tensor_tensor(var, in0=sumsq_ps, scalar=1.0 / H,
                                       in1=msq, op0=ALU.mult, op1=ALU.subtract)
        std = awork.tile([1, ntok], F32, tag="ln_std")
        nc.scalar.activation(std, var, AF.Sqrt, bias=eps_t)
        rstd = awork.tile([1, ntok], F32, tag="ln_rstd")
        nc.vector.reciprocal(rstd, std)
        negc = awork.tile([1, ntok], F32, tag="ln_negc")
        nc.vector.scalar_tensor_tensor(negc, in0=mean, scalar=-1.0,
                                       in1=rstd, op0=ALU.mult, op1=ALU.mult)
        a_bc = awork.tile([128, ntok], F32, tag="ln_abc")
        nc.gpsimd.partition_broadcast(a_bc, rstd)
        c_bc = awork.tile([128, ntok], F32, tag="ln_cbc")
        nc.gpsimd.partition_broadcast(c_bc, negc)
        for ic in range(IC):
            tmp = awork.tile([128, ntok], F32, tag="ln_tmp")
            nc.vector.tensor_mul(tmp, src[:, ic, :], a_bc)
            nc.vector.tensor_add(tmp, tmp, c_bc)
            nc.vector.tensor_scalar(out=dst[:, ic, :], in0=tmp,
                                    scalar1=g[:, ic:ic + 1], scalar2=b[:, ic:ic + 1],
                                    op0=ALU.mult, op1=ALU.add)

    def proj_fm(dst, src, w_dram_ot, bias, bias_col):
        """dst[:, ot, :] = W @ src + b  (fm in, fm out); src/W bf16."""
        ntok = src.shape[2]
        for ot in range(IC):
            wt = wpool.tile([128, IC, 128], BF16, tag="wsl")
            nc.sync.dma_start(out=wt, in_=w_dram_ot(ot))
            if DEBUG and dbg_first[0] and ot == 0:
                dbg_first[0] = False
                nc.sync.dma_start(out=dt["d_wt"], in_=wt)
                nc.sync.dma_start(out=dt["d_qnin"], in_=src)
            ps = aps_mm.tile([128, ntok], F32, tag="mm")
            for ic in range(IC):
                nc.tensor.matmul(ps, wt[:, ic, :], src[:, ic, :],
                                 start=(ic == 0), stop=(ic == IC - 1))
            nc.scalar.add(dst[:, ot, :], ps, bias[:, bias_col(ot)])

    dbg_first = [True]

    def attention(new_resid, old_resid, qsrc, kvsrc, w_in, b_in, w_out, b_out):
        """new_resid = old_resid + out_proj(MHA(q=qsrc, kv=kvsrc)); all fm.

        qsrc/kvsrc bf16; PV uses v^T as the stationary operand so the head
        outputs land directly feature-major (no post-transpose).
        """
        qf = aw1.tile([128, IC, T], BF16, tag="qf")
        kf = aw1.tile([128, IC, T], BF16, tag="kf")
        vf = aw1.tile([128, IC, T], BF16, tag="vf")
        proj_fm(qf, qsrc, lambda ot: w_in[0, ot], b_in, lambda ot: slice(0 * IC + ot, 0 * IC + ot + 1))
        proj_fm(kf, kvsrc, lambda ot: w_in[1, ot], b_in, lambda ot: slice(1 * IC + ot, 1 * IC + ot + 1))
        proj_fm(vf, kvsrc, lambda ot: w_in[2, ot], b_in, lambda ot: slice(2 * IC + ot, 2 * IC + ot + 1))
        # attention output, feature-major (PV uses v^T as stationary)
        ao = aw1.tile([128, IC, T], BF16, tag="ao")
        for pair in range(NH // 2):
            # head pair p -> feature chunk p; head 2p partitions 0-63,
            # head 2p+1 partitions 64-127 of one [128, T] psum tile
            pvp = aps_pv.tile([128, T], F32, tag="pv")
            for j in range(2):
                h = 2 * pair + j
                base = (h % 2) * HD
                c = h // 2
                qh = qf[base:base + HD, c, :]
                kh = kf[base:base + HD, c, :]
                vh = vf[base:base + HD, c, :]
                idn = identb[base:base + HD, base:base + HD]
                # vh^T : [T, HD] in two 128-token tiles
                vht = awork.tile([128, 2, HD], BF16, tag="vht")
                for kt in range(2):
                    tp = aps_tr.tile([128, HD], BF16, tag="tr")
                    nc.tensor.transpose(tp, vh[:, kt * 128:(kt + 1) * 128], idn)
                    nc.vector.tensor_copy(vht[:, kt, :], tp)
                attn_t = awork.tile([128, 2, T], BF16, tag="attnT")
                for qt in range(2):
                    sc = aps_mm.tile([128, T], F32, tag="mm")
                    nc.tensor.matmul(sc, qh[:, qt * 128:(qt + 1) * 128], kh,
                                     start=True, stop=True)
                    nmax = awork.tile([128, 1], F32, tag="nmax")
                    nc.vector.reduce_max(nmax, sc, axis=AX.X, negate=True)
                    nmax2 = awork.tile([128, 1], F32, tag="nmax2")
                    nc.scalar.mul(nmax2, nmax, 0.125)
                    asb = awork.tile([128, T], F32, tag="asb")
                    ssum = awork.tile([128, 1], F32, tag="ssum")
                    nc.scalar.activation(asb, sc, AF.Exp, bias=nmax2, scale=0.125,
                                         accum_out=ssum)
                    rsum = awork.tile([128, 1], F32, tag="rsum")
                    nc.vector.reciprocal(rsum, ssum)
                    asb_r = awork.tile([128, T], BF16, tag="asb_r")
                    nc.vector.tensor_scalar_mul(asb_r, asb, rsum)
                    for kt in range(2):
                        tp2 = aps_tr.tile([128, 128], BF16, tag="tr")
                        nc.tensor.transpose(tp2, asb_r[:, kt * 128:(kt + 1) * 128],
                                            identb)
                        nc.vector.tensor_copy(
                            attn_t[:, kt, qt * 128:(qt + 1) * 128], tp2)
                # PV: out[d, q] = vht^T @ attn_t, feature-major directly
                for kt in range(2):
                    nc.tensor.matmul(pvp[base:base + HD, :], vht[:, kt, :],
                                     attn_t[:, kt, :],
                                     start=(kt == 0), stop=(kt == 1))
            nc.vector.tensor_copy(ao[:, pair, :], pvp)
        if DEBUG and new_resid is not q2:
            nc.sync.dma_start(out=dt["d_qf"], in_=qf)
            nc.sync.dma_start(out=dt["d_ao"], in_=ao)
        # out-proj + bias + residual
        for ot in range(IC):
            wt = wpool.tile([128, IC, 128], BF16, tag="wsl")
            nc.sync.dma_start(out=wt, in_=w_out[ot])
            ps = aps_mm.tile([128, T], F32, tag="mm")
            for ic in range(IC):
                nc.tensor.matmul(ps, wt[:, ic, :], ao[:, ic, :],
                                 start=(ic == 0), stop=(ic == IC - 1))
            nc.vector.scalar_tensor_tensor(new_resid[:, ot, :], in0=ps,
                                           scalar=b_out[:, ot:ot + 1],
                                           in1=old_resid[:, ot, :],
                                           op0=ALU.add, op1=ALU.add)

    def gate(s, tokens_fm, w1sb, w2sb, bsb, ctx):
        """probs_bc[:, :, s*T:(s+1)*T] = softmax_E(tokens.W1 + ctx.W2 + b)."""
        ct_ps = aps_sm.tile([1, E], F32, tag="sm")
        for ic in range(IC):
            nc.tensor.matmul(ct_ps, ctx[:, ic, :], w2sb[:, ic, :],
                             start=(ic == 0), stop=(ic == IC - 1))
        crow = awork.tile([1, E], F32, tag="crow")
        nc.vector.tensor_add(crow, ct_ps, bsb)
        crow_bc = awork.tile([128, E], F32, tag="crow_bc")
        nc.gpsimd.partition_broadcast(crow_bc, crow)
        ptm = awork.tile([128, 2, E], F32, tag="ptm")
        for tt in range(2):
            lg_ps = aps_tr.tile([128, E], F32, tag="tr")
            for ic in range(IC):
                nc.tensor.matmul(lg_ps, tokens_fm[:, ic, tt * 128:(tt + 1) * 128],
                                 w1sb[:, ic, :],
                                 start=(ic == 0), stop=(ic == IC - 1))
            lg = awork.tile([128, E], F32, tag="lg")
            nc.vector.tensor_add(lg, lg_ps, crow_bc)
            nm = awork.tile([128, 1], F32, tag="gnm")
            nc.vector.reduce_max(nm, lg, axis=AX.X, negate=True)
            gs = awork.tile([128, 1], F32, tag="gs")
            nc.scalar.activation(ptm[:, tt, :], lg, AF.Exp, bias=nm, accum_out=gs)
            gr = awork.tile([128, 1], F32, tag="gr")
            nc.vector.reciprocal(gr, gs)
            nc.vector.tensor_scalar_mul(ptm[:, tt, :], ptm[:, tt, :], gr)
        # fold the 1/SW fp8 descale of the expert outputs into the probs
        pfm = awork.tile([E, 2, 128], F32, tag="pfm")
        for tt in range(2):
            tp = aps_tr.tile([E, 128], F32, tag="tr")
            nc.tensor.transpose(tp, ptm[:, tt, :], ident)
            nc.scalar.mul(pfm[:, tt, :], tp, 1.0 / SW)
        nc.sync.dma_start(out=scr_probs[s], in_=pfm)
        nc.sync.dma_start(out=probs_bc[:, :, s * T:(s + 1) * T],
                          in_=scr_probs[s].partition_broadcast(128))

    # ---- phase-1 body ---------------------------------------------------
    qn = aw1.tile([128, IC, T], BF16, tag="qn")
    with nc.allow_low_precision(reason="bf16 attention operands"):
        ln_fm(qn, xq0, lnp["g_lnq"], lnp["b_lnq"])

    if STAGE == 1:
        if DEBUG:
            nc.sync.dma_start(out=dt["d_qnin"], in_=qn)
        if not SKIP_TILES:
            nc.vector.memset(acc.rearrange("p a b -> p (a b)"), 0.0)
        _finish(nc, tc, dt, locals())
        return
    if STAGE == 0.5:
        # pure DVE cast copy into bf16 (no LN math)
        qc = aw1.tile([128, IC, T], BF16, tag="qc")
        with nc.allow_low_precision(reason="dbg"):
            nc.vector.tensor_copy(qc, xq0)
        # pure DMA import of bf16 weights
        wimp = aw1.tile([128, IC, 128], BF16, tag="wimp")
        nc.sync.dma_start(out=wimp, in_=dt["w_sa"][0, 0])
        if DEBUG:
            nc.sync.dma_start(out=dt["d_qnin"], in_=qc)
            nc.sync.dma_start(out=dt["d_wt"], in_=wimp)
        if not SKIP_TILES:
            nc.vector.memset(acc.rearrange("p a b -> p (a b)"), 0.0)
        _finish(nc, tc, dt, locals())
        return

    # work that depends only on xi0/xt0, issued early to fill ln/DMA stalls
    xib = persist.tile([128, IC, T], BF16, tag="xib")
    with nc.allow_low_precision(reason="bf16 attention operands / fp8 moe"):
        nc.vector.tensor_copy(xib, xi0)
        nc.scalar.mul(x8[:, :, T:T2], xi0, SX)
    nc.vector.tensor_copy(acc[:, :, T:T2], xi0)
    ictx = awork.tile([128, IC, 1], F32R, tag="ictx")
    tctx = awork.tile([128, IC, 1], F32R, tag="tctx")
    with nc.allow_low_precision(reason="f32r shares f32 bits; DVE sum is fp32"):
        for ic in range(IC):
            nc.vector.reduce_sum(ictx[:, ic, :], xi0[:, ic, :], axis=AX.X)
            nc.vector.reduce_sum(tctx[:, ic, :], xt0[:, ic, :], axis=AX.X)
    nc.scalar.mul(ictx.rearrange("p a b -> p (a b)"),
                  ictx.rearrange("p a b -> p (a b)"), 1.0 / T)
    nc.scalar.mul(tctx.rearrange("p a b -> p (a b)"),
                  tctx.rearrange("p a b -> p (a b)"), 1.0 / T)
    gate(1, xi0, w_ig1, w_ig2, b_ig, tctx)

    q1 = aw1.tile([128, IC, T], F32R, tag="q1")
    attention(q1, xq0, qn, qn, dt["w_sa"], b_sa.rearrange("p a b -> p (a b)"),
              dt["w_sao"], b_sao)
    if STAGE == 2:
        if DEBUG:
            nc.sync.dma_start(out=dt["d_q1"], in_=q1.bitcast(F32))
        if not SKIP_TILES:
            nc.vector.memset(acc.rearrange("p a b -> p (a b)"), 0.0)
        _finish(nc, tc, dt, locals())
        return

    qn2 = aw1.tile([128, IC, T], BF16, tag="qn2")
    with nc.allow_low_precision(reason="bf16 attention operands"):
        ln_fm(qn2, q1, lnp["g_lnc"], lnp["b_lnc"])
    attention(q2, q1, qn2, xib, dt["w_ca"], b_ca.rearrange("p a b -> p (a b)"),
              dt["w_cao"], b_cao)
    if STAGE == 3:
        if DEBUG:
            nc.sync.dma_start(out=dt["d_q2"], in_=q2.bitcast(F32))
        _finish(nc, tc, dt, locals())
        return

    # router for the query stream (txt gate on q2)
    gate(0, q2, w_tg1, w_tg2, b_tg, ictx)

    # moe input: SX*[ lnf(q2) ; xi0 ] quantized to fp8
    # (g_lnf/b_lnf are pre-scaled by SX on host, so ln_fm writes SX*lnf(q2))
    with nc.allow_low_precision(reason="fp8 moe input quantization"):
        ln_fm(x8[:, :, 0:T], q2, lnp["g_lnf"], lnp["b_lnf"])

    # moe accumulator initialised with the query residual
    nc.vector.tensor_copy(acc[:, :, 0:T], q2)

    if DEBUG:
        nc.sync.dma_start(out=dt["d_qn"], in_=qn)
        nc.sync.dma_start(out=dt["d_q1"], in_=q1.bitcast(F32))
        nc.sync.dma_start(out=dt["d_q2"], in_=q2.bitcast(F32))
        nc.sync.dma_start(out=dt["d_probs"], in_=probs_bc)
        nc.sync.dma_start(out=dt["d_x8"], in_=x8)

    for p in (wpool, aw1, awork, aps_sm, aps_pv, aps_tr, aps_mm):
        p.release()

    # ====================================================================
    # phase 2: dense MoE over both streams (512 tokens), fp8 DoubleRow
    # ====================================================================
    mps_h = tc.alloc_tile_pool(name="mps_h", bufs=2, space="PSUM")
    mps_o = tc.alloc_tile_pool(name="mps_o", bufs=2, space="PSUM")
    hpool = tc.alloc_tile_pool(name="hpool", bufs=2)
    mw1 = tc.alloc_tile_pool(name="mw1", bufs=4)
    mw2 = tc.alloc_tile_pool(name="mw2", bufs=2)
    mwork = tc.alloc_tile_pool(name="mwork", bufs=2)

    with nc.allow_low_precision(reason="fp8 moe"):
        for e in range(E):
            # hidden = fp8(GELU(x @ W1 + b1)); psum holds SX*SW*(x@W1)
            h_all = hpool.tile([128, FT, T2], FP8, tag="h_all")
            for ft in range(FT):
                w1t = mw1.tile([128, IC, 128], FP8, tag="w1sl")
                nc.sync.dma_start(out=w1t, in_=dt["w1"][e, ft])
                hps = mps_h.tile([128, T2], F32, tag="h")
                for c in range(IC // 2):
                    nc.tensor.matmul(hps, w1t[:, 2 * c:2 * c + 2, :],
                                     x8[:, 2 * c:2 * c + 2, :],
                                     start=(c == 0), stop=(c == IC // 2 - 1),
                                     perf_mode=PMDR)
                nc.scalar.activation(h_all[:, ft, :], hps, AF.Gelu,
                                     bias=b1f[:, e, ft:ft + 1],
                                     scale=1.0 / (SX * SW))
            # out = (h @ W2)*SW + SW*b2, mixed by probs/SW into acc
            for oc in range(IC):
                w2t = mw2.tile([128, FT, 128], FP8, tag="w2sl")
                nc.sync.dma_start(out=w2t, in_=dt["w2"][e, oc])
                ops = mps_o.tile([128, T2], F32, tag="o")
                for c in range(FT // 2):
                    nc.tensor.matmul(ops, w2t[:, 2 * c:2 * c + 2, :],
                                     h_all[:, 2 * c:2 * c + 2, :],
                                     start=(c == 0), stop=(c == FT // 2 - 1),
                                     perf_mode=PMDR)
                tmp = mwork.tile([128, T2], F32, tag="otmp")
                nc.vector.scalar_tensor_tensor(tmp, in0=ops,
                                               scalar=b2f[:, e, oc:oc + 1],
                                               in1=probs_bc[:, e, :],
                                               op0=ALU.add, op1=ALU.mult)
                nc.vector.tensor_add(acc[:, oc, :], acc[:, oc, :], tmp)

    # ---- outputs: transpose back to token-major and store ---------------
    for s, oname in ((0, "oq"), (1, "oi")):
        for tt in range(2):
            otile = mwork.tile([128, H], F32, tag="otm")
            for oc in range(IC):
                tp = mps_h.tile([128, 128], F32, tag="tp")
                nc.tensor.transpose(
                    tp, acc[:, oc, s * T + tt * 128: s * T + (tt + 1) * 128], ident)
                nc.vector.tensor_copy(otile[:, oc * 128:(oc + 1) * 128], tp)
            nc.sync.dma_start(out=dt[oname][:, tt, :], in_=otile)

    for p in (mwork, mw2, mw1, hpool, mps_o, mps_h, dram, persist):
        p.release()


# ----------------------------------------------------------------------------
# host-side prep + run
# ----------------------------------------------------------------------------

_NC = None
LAST_EXEC_NS = None


def _get_nc():
    global _NC
    if _NC is None:
        _NC = _build_program()
    return _NC


def _prep_inputs(inp):
    """Build the per-core in_maps from the full (unsharded) numpy inputs."""
    f = np.float32

    def c(a):
        return np.ascontiguousarray(a, dtype=f)

    def cb(a):  # bf16 attention weights
        return np.ascontiguousarray(a, dtype=f).astype(ml_dtypes.bfloat16)

    shared = {}
    shared["w_sa"] = cb(inp["sa_in_w"].reshape(3, 8, 128, IC, 128).transpose(0, 1, 4, 3, 2))
    shared["b_sa"] = c(inp["sa_in_b"].reshape(3, IC, 128).transpose(2, 0, 1))
    shared["w_sao"] = cb(inp["sa_out_w"].reshape(8, 128, IC, 128).transpose(0, 3, 2, 1))
    shared["b_sao"] = c(inp["sa_out_b"].reshape(IC, 128).T)
    shared["w_ca"] = cb(inp["ca_in_w"].reshape(3, 8, 128, IC, 128).transpose(0, 1, 4, 3, 2))
    shared["b_ca"] = c(inp["ca_in_b"].reshape(3, IC, 128).transpose(2, 0, 1))
    shared["w_cao"] = cb(inp["ca_out_w"].reshape(8, 128, IC, 128).transpose(0, 3, 2, 1))
    shared["b_cao"] = c(inp["ca_out_b"].reshape(IC, 128).T)
    shared["w_ig1"] = c(inp["img_gate_w"][:, :H].T.reshape(IC, 128, E).transpose(1, 0, 2))
    shared["w_ig2"] = c(inp["img_gate_w"][:, H:].T.reshape(IC, 128, E).transpose(1, 0, 2))
    shared["w_tg1"] = c(inp["txt_gate_w"][:, :H].T.reshape(IC, 128, E).transpose(1, 0, 2))
    shared["w_tg2"] = c(inp["txt_gate_w"][:, H:].T.reshape(IC, 128, E).transpose(1, 0, 2))
    shared["b_g"] = c(np.broadcast_to(
        np.stack([np.asarray(inp["img_gate_b"]), np.asarray(inp["txt_gate_b"])]),
        (128, 2, E)))
    for n, k in (("g_lnq", "lnq_g"), ("b_lnq", "lnq_b"), ("g_lnc", "lnc_g"),
                 ("b_lnc", "lnc_b"), ("g_lnf", "lnf_g"), ("b_lnf", "lnf_b")):
        shared[n] = c(inp[k].reshape(IC, 128).T)
    # lnf output is quantized to fp8 as SX*lnf(x): fold SX into gamma/beta
    shared["g_lnf"] = shared["g_lnf"] * np.float32(SX)
    shared["b_lnf"] = shared["b_lnf"] * np.float32(SX)

    def q8(a, s):
        return np.clip(np.asarray(a, np.float32) * np.float32(s),
                       -240.0, 240.0).astype(ml_dtypes.float8_e4m3)

    shared["w1"] = q8(np.asarray(inp["ew1"]).reshape(E, IC, 128, FT, 128)
                      .transpose(0, 3, 2, 1, 4), SW)
    shared["b1"] = c(inp["eb1"].reshape(E, FT, 128).transpose(2, 0, 1))
    shared["w2"] = q8(np.asarray(inp["ew2"]).reshape(E, FT, 128, IC, 128)
                      .transpose(0, 3, 2, 1, 4), SW)
    shared["b2"] = c(inp["eb2"].reshape(E, IC, 128).transpose(2, 0, 1)) * np.float32(SW)

    def fm(a):  # [T, H] -> [128, IC, T]
        return c(a.T.reshape(IC, 128, T).transpose(1, 0, 2))

    in_maps = []
    for b in range(B):
        m = dict(shared)
        m["xq"] = fm(np.asarray(inp["query_tokens"][b]))
        m["xi"] = fm(np.asarray(inp["image_tokens"][b]))
        m["xt"] = fm(np.asarray(inp["text_context"][b]))
        in_maps.append(m)
    return in_maps


def _run(inp, trace=False):
    global LAST_EXEC_NS
    nc = _get_nc()
    in_maps = _prep_inputs(inp)
    res = run_bass_kernel_spmd(nc, in_maps, core_ids=list(range(B)), trace=trace)
    LAST_EXEC_NS = res.exec_time_ns
    oq = np.empty((B, T, H), np.float32)
    oi = np.empty((B, T, H), np.float32)
    for b in range(B):
        oq[b] = res.results[b]["oq"].transpose(1, 0, 2).reshape(T, H)
        oi[b] = res.results[b]["oi"].transpose(1, 0, 2).reshape(T, H)
    return oq, oi


def kernel(**inputs):
    return _run(inputs, trace=False)



# revision 56
# speedup vs baseline: 2.1615x; 1.0667x over previous
"""CrossModalMoELayer Trainium2 Bass kernel.

Sharding: data-parallel over batch B=8 across the 8 NeuronCores (one batch
element per core). Each core runs the full layer for its batch element:
self-attention, cross-attention, gating, and the dense 8-expert MoE
(weights replicated, streamed from HBM).

Layouts on device:
  feature-major ("fm"): [feat_part=128, feat_chunk, tokens]  - activations
  All matmuls run as float32r (full-rate fp32 PE mode, fp32 PSUM accumulate).

kernel(**inputs) takes the FULL unsharded inputs (numpy, keyed as in
setup_inputs()) and returns the full (query_tokens, image_tokens) tuple.
"""

import ml_dtypes
import numpy as np

import concourse.bass as bass
import concourse.tile as tile
from concourse import bacc, mybir
from concourse.bass_utils import run_bass_kernel_spmd
from concourse.masks import make_identity

DEBUG = False
STAGE = 5
SKIP_LOADS = set()     # load names to skip (debug bisection)
SKIP_TILES = False     # skip q2/x8/probs/acc persist tiles (debug bisection)
B, T, H, NH, HD, F, E = 8, 256, 1024, 16, 64, 4096, 8
IC = H // 128          # 8 feature chunks of the model dim
FT = F // 128          # 32 feature chunks of the FFN dim
T2 = 2 * T             # query tokens + image tokens concatenated
EPS = 1e-5

F32 = mybir.dt.float32
F32R = mybir.dt.float32r
BF16 = mybir.dt.bfloat16
FP8 = mybir.dt.float8e4
PMDR = mybir.MatmulPerfMode.DoubleRow
AX = mybir.AxisListType
ALU = mybir.AluOpType
AF = mybir.ActivationFunctionType

# fp8 quantization scales: x8 = fp8(SX * x), w = fp8(SW * w_fp32)
SX = 8.0
SW = 512.0


def _r(ap):
    return ap.bitcast(F32R)


# ----------------------------------------------------------------------------
# program builder
# ----------------------------------------------------------------------------

def _build_program():
    nc = bacc.Bacc(
        "TRN2",
        target_bir_lowering=False,
        debug=False,
        enable_asserts=False,
        num_devices=8,
    )

    dt = {}

    def din(name, shape, d=F32):
        dt[name] = nc.dram_tensor(name, list(shape), d, kind="ExternalInput").ap()
        return dt[name]

    def dout(name, shape):
        dt[name] = nc.dram_tensor(name, list(shape), F32, kind="ExternalOutput").ap()
        return dt[name]

    # activations (per core)
    din("xq", (128, IC, T), F32R)
    din("xi", (128, IC, T), F32R)
    din("xt", (128, IC, T), F32R)
    # attention weights: [proj, ot, i(128), ic, o(128)] (bf16)
    din("w_sa", (3, 8, 128, IC, 128), BF16)
    din("b_sa", (128, 3, IC))
    din("w_sao", (8, 128, IC, 128), BF16)
    din("b_sao", (128, IC))
    din("w_ca", (3, 8, 128, IC, 128), BF16)
    din("b_ca", (128, 3, IC))
    din("w_cao", (8, 128, IC, 128), BF16)
    din("b_cao", (128, IC))
    # gates (biases packed in one 128-partition tensor: 1-partition DMAs
    # corrupt the low 16 bits of unrelated SBUF words on partitions 64-127)
    din("w_ig1", (128, IC, E), F32R)
    din("w_ig2", (128, IC, E), F32R)
    din("w_tg1", (128, IC, E), F32R)
    din("w_tg2", (128, IC, E), F32R)
    din("b_g", (128, 2, E))
    # layernorms [128, IC]
    for n in ("g_lnq", "b_lnq", "g_lnc", "b_lnc", "g_lnf", "b_lnf"):
        din(n, (128, IC))
    # experts (fp8, pre-scaled by SW on host)
    din("w1", (E, FT, 128, IC, 128), FP8)      # [e, ft, i, ic, f]
    din("b1", (128, E, FT))
    din("w2", (E, IC, 128, FT, 128), FP8)      # [e, oc, i(f%128), ft, o]
    din("b2", (128, E, IC))
    # output: the MoE accumulator in feature-major layout (host de-interleaves)
    dout("oacc", (128, IC, T2))
    if DEBUG:
        dt["d_qn"] = nc.dram_tensor("d_qn", [128, IC, T], BF16, kind="ExternalOutput").ap()
        dt["d_qf"] = nc.dram_tensor("d_qf", [128, IC, T], BF16, kind="ExternalOutput").ap()
        dt["d_wt"] = nc.dram_tensor("d_wt", [128, IC, 128], BF16, kind="ExternalOutput").ap()
        dt["d_qnin"] = nc.dram_tensor("d_qnin", [128, IC, T], BF16, kind="ExternalOutput").ap()
        dt["d_ao"] = nc.dram_tensor("d_ao", [128, IC, T], BF16, kind="ExternalOutput").ap()
        dt["d_q1"] = nc.dram_tensor("d_q1", [128, IC, T], F32, kind="ExternalOutput").ap()
        dt["d_q2"] = nc.dram_tensor("d_q2", [128, IC, T], F32, kind="ExternalOutput").ap()
        dt["d_probs"] = nc.dram_tensor("d_probs", [128, E, T2], F32, kind="ExternalOutput").ap()
        dt["d_x8"] = nc.dram_tensor("d_x8", [128, IC, T2], FP8, kind="ExternalOutput").ap()

    with tile.TileContext(nc) as tc:
        _trace_kernel(nc, tc, dt)

    nc.compile()
    return nc


def _finish(nc, tc, dt, l):
    """Early-exit for STAGE bisection: dummy outputs + pool release."""
    out0 = l["persist"].tile([128, IC, T2], F32, tag="dummy_out")
    nc.vector.memset(out0.rearrange("p a b -> p (a b)"), 0.0)
    nc.sync.dma_start(out=dt["oacc"], in_=out0)
    for name in ("wpool", "aw1", "awork", "aps_sm", "aps_pv", "aps_tr",
                 "aps_mm", "dram", "persist"):
        if name in l:
            l[name].release()


def _trace_kernel(nc, tc, dt):
    persist = tc.alloc_tile_pool(name="persist", bufs=1)

    def load(name, shape, d=F32, pool=persist):
        t = pool.tile(list(shape), d, tag=f"ld_{name}")
        if name not in SKIP_LOADS:
            nc.sync.dma_start(out=t, in_=dt[name])
        else:
            ap = t if len(shape) <= 2 else t.rearrange("p a b -> p (a b)")
            nc.vector.memset(ap.bitcast(F32) if d == F32R else ap, 0.0)
        return t

    # critical-path activations first so phase-1 compute starts ASAP
    # (split into halves so the first LN matmuls start mid-transfer)
    xq0 = persist.tile([128, IC, T], F32R, tag="xq0")
    nc.sync.dma_start(out=xq0[:, 0:IC // 2, :], in_=dt["xq"][:, 0:IC // 2, :])
    nc.sync.dma_start(out=xq0[:, IC // 2:IC, :], in_=dt["xq"][:, IC // 2:IC, :])
    lnp = {n: load(n, (128, IC)) for n in
           ("g_lnq", "b_lnq", "g_lnc", "b_lnc", "g_lnf", "b_lnf")}
    xi0 = load("xi", (128, IC, T), F32R)
    xt0 = load("xt", (128, IC, T), F32R)

    # ---- constants + small params --------------------------------------
    ident = persist.tile([128, 128], F32, tag="ident")
    make_identity(nc, ident)
    ones_f = persist.tile([128, 1], F32, tag="ones_f")
    nc.vector.memset(ones_f, 1.0)
    ones = persist.tile([128, 1], F32R, tag="ones")
    nc.vector.tensor_copy(ones, ones_f)
    identb = persist.tile([128, 128], BF16, tag="identb")
    nc.vector.tensor_copy(identb, ident)
    eps_t = persist.tile([1, 1], F32, tag="eps")
    nc.vector.memset(eps_t, EPS)

    b_sa = load("b_sa", (128, 3, IC))
    b_sao = load("b_sao", (128, IC))
    b_ca = load("b_ca", (128, 3, IC))
    b_cao = load("b_cao", (128, IC))
    w_ig1 = load("w_ig1", (128, IC, E), F32R)
    w_ig2 = load("w_ig2", (128, IC, E), F32R)
    w_tg1 = load("w_tg1", (128, IC, E), F32R)
    w_tg2 = load("w_tg2", (128, IC, E), F32R)
    b_g = load("b_g", (128, 2, E))
    b_ig = b_g[0:1, 0, :]
    b_tg = b_g[0:1, 1, :]
    b1f = load("b1", (128, E, FT))
    b2f = load("b2", (128, E, IC))

    # persistent activations
    if not SKIP_TILES:
        q2 = persist.tile([128, IC, T], F32R, tag="q2")          # query after CA
        x8 = persist.tile([128, IC, T2], FP8, tag="x8")          # SX*[lnf(q2) ; xi0]
        probs_bc = persist.tile([128, E, T2], F32, tag="probs") # router probs bcast
        acc = persist.tile([128, IC, T2], F32, tag="acc")       # MoE accumulator

        dram = tc.alloc_tile_pool(name="dram", bufs=1, space="DRAM")
        scr_probs = dram.tile([2, E, T], F32, tag="scr_probs")

    # ====================================================================
    # phase 1: attention + gating + lnf (own pools, released before MoE)
    # ====================================================================
    aps_mm = tc.alloc_tile_pool(name="aps_mm", bufs=3, space="PSUM")
    aps_tr = tc.alloc_tile_pool(name="aps_tr", bufs=2, space="PSUM")
    aps_pv = tc.alloc_tile_pool(name="aps_pv", bufs=2, space="PSUM")
    aps_sm = tc.alloc_tile_pool(name="aps_sm", bufs=1, space="PSUM")
    awork = tc.alloc_tile_pool(name="awork", bufs=2)
    aw1 = tc.alloc_tile_pool(name="aw1", bufs=1)
    wpool = tc.alloc_tile_pool(name="wpool", bufs=6)

    def ln_fm(dst, src, g, b):
        """dst[:, ic, :] = LN over features of src (fm layout [128, IC, T])."""
        ntok = src.shape[2]
        sum_ps = aps_sm.tile([1, ntok], F32, tag="sm")
        for ic in range(IC):
            nc.tensor.matmul(sum_ps, ones, src[:, ic, :],
                             start=(ic == 0), stop=(ic == IC - 1))
        mean = awork.tile([1, ntok], F32, tag="ln_mean")
        nc.scalar.mul(mean, sum_ps, 1.0 / H)
        sumsq_ps = aps_sm.tile([1, ntok], F32, tag="sm")
        for ic in range(IC):
            xsq = awork.tile([128, ntok], F32R, tag="ln_xsq")
            nc.scalar.activation(xsq, src[:, ic, :], AF.Square)
            nc.tensor.matmul(sumsq_ps, ones, xsq,
                             start=(ic == 0), stop=(ic == IC - 1))
        msq = awork.tile([1, ntok], F32, tag="ln_msq")
        nc.vector.tensor_mul(msq, mean, mean)
        var = awork.tile([1, ntok], F32, tag="ln_var")
        nc.vector.scalar_tensor_tensor(var, in0=sumsq_ps, scalar=1.0 / H,
                                       in1=msq, op0=ALU.mult, op1=ALU.subtract)
        std = awork.tile([1, ntok], F32, tag="ln_std")
        nc.scalar.activation(std, var, AF.Sqrt, bias=eps_t)
        rstd = awork.tile([1, ntok], F32, tag="ln_rstd")
        nc.vector.reciprocal(rstd, std)
        negc = awork.tile([1, ntok], F32, tag="ln_negc")
        nc.vector.scalar_tensor_tensor(negc, in0=mean, scalar=-1.0,
                                       in1=rstd, op0=ALU.mult, op1=ALU.mult)
        a_bc = awork.tile([128, ntok], F32, tag="ln_abc")
        nc.gpsimd.partition_broadcast(a_bc, rstd)
        c_bc = awork.tile([128, ntok], F32, tag="ln_cbc")
        nc.gpsimd.partition_broadcast(c_bc, negc)
        for ic in range(IC):
            tmp = awork.tile([128, ntok], F32, tag="ln_tmp")
            nc.vector.tensor_mul(tmp, src[:, ic, :], a_bc)
            nc.vector.tensor_add(tmp, tmp, c_bc)
            nc.vector.tensor_scalar(out=dst[:, ic, :], in0=tmp,
                                    scalar1=g[:, ic:ic + 1], scalar2=b[:, ic:ic + 1],
                                    op0=ALU.mult, op1=ALU.add)

    def proj_fm(dst, src, w_dram_ot, bias, bias_col):
        """dst[:, ot, :] = W @ src + b  (fm in, fm out); src/W bf16."""
        ntok = src.shape[2]
        for ot in range(IC):
            wt = wpool.tile([128, IC, 128], BF16, tag="wsl")
            nc.sync.dma_start(out=wt, in_=w_dram_ot(ot))
            if DEBUG and dbg_first[0] and ot == 0:
                dbg_first[0] = False
                nc.sync.dma_start(out=dt["d_wt"], in_=wt)
                nc.sync.dma_start(out=dt["d_qnin"], in_=src)
            ps = aps_mm.tile([128, ntok], F32, tag="mm")
            for ic in range(IC):
                nc.tensor.matmul(ps, wt[:, ic, :], src[:, ic, :],
                                 start=(ic == 0), stop=(ic == IC - 1))
            nc.scalar.add(dst[:, ot, :], ps, bias[:, bias_col(ot)])

    dbg_first = [True]

    def attention(new_resid, old_resid, qsrc, kvsrc, w_in, b_in, w_out, b_out,
                  kv_first=False):
        """new_resid = old_resid + out_proj(MHA(q=qsrc, kv=kvsrc)); all fm.

        qsrc/kvsrc bf16; PV uses v^T as the stationary operand so the head
        outputs land directly feature-major (no post-transpose). kv_first
        issues the K/V projections before Q (when kvsrc is ready earlier
        than qsrc, PE stays busy during the preceding layernorm chain).
        """
        qf = aw1.tile([128, IC, T], BF16, tag="qf")
        kf = aw1.tile([128, IC, T], BF16, tag="kf")
        vf = aw1.tile([128, IC, T], BF16, tag="vf")
        projs = [
            (qf, qsrc, 0),
            (kf, kvsrc, 1),
            (vf, kvsrc, 2),
        ]
        if kv_first:
            projs = projs[1:] + projs[:1]
        for dst_, src_, pj in projs:
            proj_fm(dst_, src_, lambda ot, pj=pj: w_in[pj, ot], b_in,
                    lambda ot, pj=pj: slice(pj * IC + ot, pj * IC + ot + 1))
        # attention output, feature-major (PV uses v^T as stationary)
        ao = aw1.tile([128, IC, T], BF16, tag="ao")
        for pair in range(NH // 2):
            # head pair p -> feature chunk p; head 2p partitions 0-63,
            # head 2p+1 partitions 64-127 of one [128, T] psum tile
            pvp = aps_pv.tile([128, T], F32, tag="pv")
            for j in range(2):
                h = 2 * pair + j
                base = (h % 2) * HD
                c = h // 2
                qh = qf[base:base + HD, c, :]
                kh = kf[base:base + HD, c, :]
                vh = vf[base:base + HD, c, :]
                idn = identb[base:base + HD, base:base + HD]
                # vh^T : [T, HD] in two 128-token tiles
                vht = awork.tile([128, 2, HD], BF16, tag="vht")
                for kt in range(2):
                    tp = aps_tr.tile([128, HD], BF16, tag="tr")
                    nc.tensor.transpose(tp, vh[:, kt * 128:(kt + 1) * 128], idn)
                    nc.vector.tensor_copy(vht[:, kt, :], tp)
                attn_t = awork.tile([128, 2, T], BF16, tag="attnT")
                for qt in range(2):
                    sc = aps_mm.tile([128, T], F32, tag="mm")
                    nc.tensor.matmul(sc, qh[:, qt * 128:(qt + 1) * 128], kh,
                                     start=True, stop=True)
                    nmax = awork.tile([128, 1], F32, tag="nmax")
                    nc.vector.reduce_max(nmax, sc, axis=AX.X, negate=True)
                    nmax2 = awork.tile([128, 1], F32, tag="nmax2")
                    nc.scalar.mul(nmax2, nmax, 0.125)
                    asb = awork.tile([128, T], F32, tag="asb")
                    ssum = awork.tile([128, 1], F32, tag="ssum")
                    nc.scalar.activation(asb, sc, AF.Exp, bias=nmax2, scale=0.125,
                                         accum_out=ssum)
                    rsum = awork.tile([128, 1], F32, tag="rsum")
                    nc.vector.reciprocal(rsum, ssum)
                    asb_r = awork.tile([128, T], BF16, tag="asb_r")
                    nc.vector.tensor_scalar_mul(asb_r, asb, rsum)
                    for kt in range(2):
                        tp2 = aps_tr.tile([128, 128], BF16, tag="tr")
                        nc.tensor.transpose(tp2, asb_r[:, kt * 128:(kt + 1) * 128],
                                            identb)
                        nc.vector.tensor_copy(
                            attn_t[:, kt, qt * 128:(qt + 1) * 128], tp2)
                # PV: out[d, q] = vht^T @ attn_t, feature-major directly
                for kt in range(2):
                    nc.tensor.matmul(pvp[base:base + HD, :], vht[:, kt, :],
                                     attn_t[:, kt, :],
                                     start=(kt == 0), stop=(kt == 1))
            nc.vector.tensor_copy(ao[:, pair, :], pvp)
        if DEBUG and new_resid is not q2:
            nc.sync.dma_start(out=dt["d_qf"], in_=qf)
            nc.sync.dma_start(out=dt["d_ao"], in_=ao)
        # out-proj + bias + residual
        for ot in range(IC):
            wt = wpool.tile([128, IC, 128], BF16, tag="wsl")
            nc.sync.dma_start(out=wt, in_=w_out[ot])
            ps = aps_mm.tile([128, T], F32, tag="mm")
            for ic in range(IC):
                nc.tensor.matmul(ps, wt[:, ic, :], ao[:, ic, :],
                                 start=(ic == 0), stop=(ic == IC - 1))
            nc.vector.scalar_tensor_tensor(new_resid[:, ot, :], in0=ps,
                                           scalar=b_out[:, ot:ot + 1],
                                           in1=old_resid[:, ot, :],
                                           op0=ALU.add, op1=ALU.add)

    def gate(s, tokens_fm, w1sb, w2sb, bsb, ctx):
        """probs_bc[:, :, s*T:(s+1)*T] = softmax_E(tokens.W1 + ctx.W2 + b)."""
        ct_ps = aps_sm.tile([1, E], F32, tag="sm")
        for ic in range(IC):
            nc.tensor.matmul(ct_ps, ctx[:, ic, :], w2sb[:, ic, :],
                             start=(ic == 0), stop=(ic == IC - 1))
        crow = awork.tile([1, E], F32, tag="crow")
        nc.vector.tensor_add(crow, ct_ps, bsb)
        crow_bc = awork.tile([128, E], F32, tag="crow_bc")
        nc.gpsimd.partition_broadcast(crow_bc, crow)
        ptm = awork.tile([128, 2, E], F32, tag="ptm")
        for tt in range(2):
            lg_ps = aps_tr.tile([128, E], F32, tag="tr")
            for ic in range(IC):
                nc.tensor.matmul(lg_ps, tokens_fm[:, ic, tt * 128:(tt + 1) * 128],
                                 w1sb[:, ic, :],
                                 start=(ic == 0), stop=(ic == IC - 1))
            lg = awork.tile([128, E], F32, tag="lg")
            nc.vector.tensor_add(lg, lg_ps, crow_bc)
            nm = awork.tile([128, 1], F32, tag="gnm")
            nc.vector.reduce_max(nm, lg, axis=AX.X, negate=True)
            gs = awork.tile([128, 1], F32, tag="gs")
            nc.scalar.activation(ptm[:, tt, :], lg, AF.Exp, bias=nm, accum_out=gs)
            gr = awork.tile([128, 1], F32, tag="gr")
            nc.vector.reciprocal(gr, gs)
            nc.vector.tensor_scalar_mul(ptm[:, tt, :], ptm[:, tt, :], gr)
        # fold the 1/SW fp8 descale of the expert outputs into the probs
        pfm = awork.tile([E, 2, 128], F32, tag="pfm")
        for tt in range(2):
            tp = aps_tr.tile([E, 128], F32, tag="tr")
            nc.tensor.transpose(tp, ptm[:, tt, :], ident)
            nc.scalar.mul(pfm[:, tt, :], tp, 1.0 / SW)
        nc.sync.dma_start(out=scr_probs[s], in_=pfm)
        nc.sync.dma_start(out=probs_bc[:, :, s * T:(s + 1) * T],
                          in_=scr_probs[s].partition_broadcast(128))

    # ---- phase-1 body ---------------------------------------------------
    qn = aw1.tile([128, IC, T], BF16, tag="qn")
    with nc.allow_low_precision(reason="bf16 attention operands"):
        ln_fm(qn, xq0, lnp["g_lnq"], lnp["b_lnq"])

    if STAGE == 1:
        if DEBUG:
            nc.sync.dma_start(out=dt["d_qnin"], in_=qn)
        if not SKIP_TILES:
            nc.vector.memset(acc.rearrange("p a b -> p (a b)"), 0.0)
        _finish(nc, tc, dt, locals())
        return
    if STAGE == 0.5:
        # pure DVE cast copy into bf16 (no LN math)
        qc = aw1.tile([128, IC, T], BF16, tag="qc")
        with nc.allow_low_precision(reason="dbg"):
            nc.vector.tensor_copy(qc, xq0)
        # pure DMA import of bf16 weights
        wimp = aw1.tile([128, IC, 128], BF16, tag="wimp")
        nc.sync.dma_start(out=wimp, in_=dt["w_sa"][0, 0])
        if DEBUG:
            nc.sync.dma_start(out=dt["d_qnin"], in_=qc)
            nc.sync.dma_start(out=dt["d_wt"], in_=wimp)
        if not SKIP_TILES:
            nc.vector.memset(acc.rearrange("p a b -> p (a b)"), 0.0)
        _finish(nc, tc, dt, locals())
        return

    # work that depends only on xi0/xt0, issued early to fill ln/DMA stalls
    xib = persist.tile([128, IC, T], BF16, tag="xib")
    with nc.allow_low_precision(reason="bf16 attention operands / fp8 moe"):
        nc.vector.tensor_copy(xib, xi0)
        nc.scalar.mul(x8[:, :, T:T2], xi0, SX)
    nc.vector.tensor_copy(acc[:, :, T:T2], xi0)
    ictx = awork.tile([128, IC, 1], F32R, tag="ictx")
    tctx = awork.tile([128, IC, 1], F32R, tag="tctx")
    with nc.allow_low_precision(reason="f32r shares f32 bits; DVE sum is fp32"):
        for ic in range(IC):
            nc.vector.reduce_sum(ictx[:, ic, :], xi0[:, ic, :], axis=AX.X)
            nc.vector.reduce_sum(tctx[:, ic, :], xt0[:, ic, :], axis=AX.X)
    nc.scalar.mul(ictx.rearrange("p a b -> p (a b)"),
                  ictx.rearrange("p a b -> p (a b)"), 1.0 / T)
    nc.scalar.mul(tctx.rearrange("p a b -> p (a b)"),
                  tctx.rearrange("p a b -> p (a b)"), 1.0 / T)
    gate(1, xi0, w_ig1, w_ig2, b_ig, tctx)

    q1 = aw1.tile([128, IC, T], F32R, tag="q1")
    attention(q1, xq0, qn, qn, dt["w_sa"], b_sa.rearrange("p a b -> p (a b)"),
              dt["w_sao"], b_sao)
    if STAGE == 2:
        if DEBUG:
            nc.sync.dma_start(out=dt["d_q1"], in_=q1.bitcast(F32))
        if not SKIP_TILES:
            nc.vector.memset(acc.rearrange("p a b -> p (a b)"), 0.0)
        _finish(nc, tc, dt, locals())
        return

    qn2 = aw1.tile([128, IC, T], BF16, tag="qn2")
    with nc.allow_low_precision(reason="bf16 attention operands"):
        ln_fm(qn2, q1, lnp["g_lnc"], lnp["b_lnc"])
    attention(q2, q1, qn2, xib, dt["w_ca"], b_ca.rearrange("p a b -> p (a b)"),
              dt["w_cao"], b_cao, kv_first=True)
    if STAGE == 3:
        if DEBUG:
            nc.sync.dma_start(out=dt["d_q2"], in_=q2.bitcast(F32))
        _finish(nc, tc, dt, locals())
        return

    # router for the query stream (txt gate on q2)
    gate(0, q2, w_tg1, w_tg2, b_tg, ictx)

    # moe input: SX*[ lnf(q2) ; xi0 ] quantized to fp8
    # (g_lnf/b_lnf are pre-scaled by SX on host, so ln_fm writes SX*lnf(q2))
    with nc.allow_low_precision(reason="fp8 moe input quantization"):
        ln_fm(x8[:, :, 0:T], q2, lnp["g_lnf"], lnp["b_lnf"])

    # moe accumulator initialised with the query residual
    nc.vector.tensor_copy(acc[:, :, 0:T], q2)

    if DEBUG:
        nc.sync.dma_start(out=dt["d_qn"], in_=qn)
        nc.sync.dma_start(out=dt["d_q1"], in_=q1.bitcast(F32))
        nc.sync.dma_start(out=dt["d_q2"], in_=q2.bitcast(F32))
        nc.sync.dma_start(out=dt["d_probs"], in_=probs_bc)
        nc.sync.dma_start(out=dt["d_x8"], in_=x8)

    for p in (wpool, aw1, awork, aps_sm, aps_pv, aps_tr, aps_mm):
        p.release()

    # ====================================================================
    # phase 2: dense MoE over both streams (512 tokens), fp8 DoubleRow
    # ====================================================================
    mps_h = tc.alloc_tile_pool(name="mps_h", bufs=3, space="PSUM")
    mps_o = tc.alloc_tile_pool(name="mps_o", bufs=2, space="PSUM")
    hpool = tc.alloc_tile_pool(name="hpool", bufs=2)
    mw1 = tc.alloc_tile_pool(name="mw1", bufs=6)
    mw2 = tc.alloc_tile_pool(name="mw2", bufs=3)
    mwork = tc.alloc_tile_pool(name="mwork", bufs=2)

    with nc.allow_low_precision(reason="fp8 moe"):
        for e in range(E):
            # hidden = fp8(GELU(x @ W1 + b1)); psum holds SX*SW*(x@W1)
            h_all = hpool.tile([128, FT, T2], FP8, tag="h_all")
            for ft in range(FT):
                w1t = mw1.tile([128, IC, 128], FP8, tag="w1sl")
                nc.sync.dma_start(out=w1t, in_=dt["w1"][e, ft])
                hps = mps_h.tile([128, T2], F32, tag="h")
                for c in range(IC // 2):
                    nc.tensor.matmul(hps, w1t[:, 2 * c:2 * c + 2, :],
                                     x8[:, 2 * c:2 * c + 2, :],
                                     start=(c == 0), stop=(c == IC // 2 - 1),
                                     perf_mode=PMDR)
                nc.scalar.activation(h_all[:, ft, :], hps, AF.Gelu,
                                     bias=b1f[:, e, ft:ft + 1],
                                     scale=1.0 / (SX * SW))
            # out = (h @ W2)*SW + SW*b2, mixed by probs/SW into acc
            for oc in range(IC):
                w2t = mw2.tile([128, FT, 128], FP8, tag="w2sl")
                nc.sync.dma_start(out=w2t, in_=dt["w2"][e, oc])
                ops = mps_o.tile([128, T2], F32, tag="o")
                for c in range(FT // 2):
                    nc.tensor.matmul(ops, w2t[:, 2 * c:2 * c + 2, :],
                                     h_all[:, 2 * c:2 * c + 2, :],
                                     start=(c == 0), stop=(c == FT // 2 - 1),
                                     perf_mode=PMDR)
                tmp = mwork.tile([128, T2], F32, tag="otmp")
                nc.vector.scalar_tensor_tensor(tmp, in0=ops,
                                               scalar=b2f[:, e, oc:oc + 1],
                                               in1=probs_bc[:, e, :],
                                               op0=ALU.add, op1=ALU.mult)
                nc.vector.tensor_add(acc[:, oc, :], acc[:, oc, :], tmp)

    # ---- output: store the accumulator directly (feature-major) ---------
    nc.sync.dma_start(out=dt["oacc"], in_=acc)

    for p in (mwork, mw2, mw1, hpool, mps_o, mps_h, dram, persist):
        p.release()


# ----------------------------------------------------------------------------
# host-side prep + run
# ----------------------------------------------------------------------------

_NC = None
LAST_EXEC_NS = None


def _get_nc():
    global _NC
    if _NC is None:
        _NC = _build_program()
    return _NC


def _prep_inputs(inp):
    """Build the per-core in_maps from the full (unsharded) numpy inputs."""
    f = np.float32

    def c(a):
        return np.ascontiguousarray(a, dtype=f)

    def cb(a):  # bf16 attention weights
        return np.ascontiguousarray(a, dtype=f).astype(ml_dtypes.bfloat16)

    shared = {}
    shared["w_sa"] = cb(inp["sa_in_w"].reshape(3, 8, 128, IC, 128).transpose(0, 1, 4, 3, 2))
    shared["b_sa"] = c(inp["sa_in_b"].reshape(3, IC, 128).transpose(2, 0, 1))
    shared["w_sao"] = cb(inp["sa_out_w"].reshape(8, 128, IC, 128).transpose(0, 3, 2, 1))
    shared["b_sao"] = c(inp["sa_out_b"].reshape(IC, 128).T)
    shared["w_ca"] = cb(inp["ca_in_w"].reshape(3, 8, 128, IC, 128).transpose(0, 1, 4, 3, 2))
    shared["b_ca"] = c(inp["ca_in_b"].reshape(3, IC, 128).transpose(2, 0, 1))
    shared["w_cao"] = cb(inp["ca_out_w"].reshape(8, 128, IC, 128).transpose(0, 3, 2, 1))
    shared["b_cao"] = c(inp["ca_out_b"].reshape(IC, 128).T)
    shared["w_ig1"] = c(inp["img_gate_w"][:, :H].T.reshape(IC, 128, E).transpose(1, 0, 2))
    shared["w_ig2"] = c(inp["img_gate_w"][:, H:].T.reshape(IC, 128, E).transpose(1, 0, 2))
    shared["w_tg1"] = c(inp["txt_gate_w"][:, :H].T.reshape(IC, 128, E).transpose(1, 0, 2))
    shared["w_tg2"] = c(inp["txt_gate_w"][:, H:].T.reshape(IC, 128, E).transpose(1, 0, 2))
    shared["b_g"] = c(np.broadcast_to(
        np.stack([np.asarray(inp["img_gate_b"]), np.asarray(inp["txt_gate_b"])]),
        (128, 2, E)))
    for n, k in (("g_lnq", "lnq_g"), ("b_lnq", "lnq_b"), ("g_lnc", "lnc_g"),
                 ("b_lnc", "lnc_b"), ("g_lnf", "lnf_g"), ("b_lnf", "lnf_b")):
        shared[n] = c(inp[k].reshape(IC, 128).T)
    # lnf output is quantized to fp8 as SX*lnf(x): fold SX into gamma/beta
    shared["g_lnf"] = shared["g_lnf"] * np.float32(SX)
    shared["b_lnf"] = shared["b_lnf"] * np.float32(SX)

    def q8(a, s):
        return np.clip(np.asarray(a, np.float32) * np.float32(s),
                       -240.0, 240.0).astype(ml_dtypes.float8_e4m3)

    shared["w1"] = q8(np.asarray(inp["ew1"]).reshape(E, IC, 128, FT, 128)
                      .transpose(0, 3, 2, 1, 4), SW)
    shared["b1"] = c(inp["eb1"].reshape(E, FT, 128).transpose(2, 0, 1))
    shared["w2"] = q8(np.asarray(inp["ew2"]).reshape(E, FT, 128, IC, 128)
                      .transpose(0, 3, 2, 1, 4), SW)
    shared["b2"] = c(inp["eb2"].reshape(E, IC, 128).transpose(2, 0, 1)) * np.float32(SW)

    def fm(a):  # [T, H] -> [128, IC, T]
        return c(a.T.reshape(IC, 128, T).transpose(1, 0, 2))

    in_maps = []
    for b in range(B):
        m = dict(shared)
        m["xq"] = fm(np.asarray(inp["query_tokens"][b]))
        m["xi"] = fm(np.asarray(inp["image_tokens"][b]))
        m["xt"] = fm(np.asarray(inp["text_context"][b]))
        in_maps.append(m)
    return in_maps


def _run(inp, trace=False):
    global LAST_EXEC_NS
    nc = _get_nc()
    in_maps = _prep_inputs(inp)
    res = run_bass_kernel_spmd(nc, in_maps, core_ids=list(range(B)), trace=trace)
    LAST_EXEC_NS = res.exec_time_ns
    oq = np.empty((B, T, H), np.float32)
    oi = np.empty((B, T, H), np.float32)
    for b in range(B):
        a = res.results[b]["oacc"]  # [128, IC, T2] feature-major
        oq[b] = a[:, :, 0:T].transpose(2, 1, 0).reshape(T, H)
        oi[b] = a[:, :, T:T2].transpose(2, 1, 0).reshape(T, H)
    return oq, oi


def kernel(**inputs):
    return _run(inputs, trace=False)

